# revision 21
# baseline (speedup 1.0000x reference)
"""MQA attention (B=2, Lq=Lkv=2048, F=1024, H=16, D=64) on 8 TRN2 cores.

Sharding: core = (batch, query-block-of-512). Each core computes its full
output rows (all 16 heads + output projection) -> no collectives; host
concatenates per-core yT slabs.

Per-core dataflow (matmuls in f32r = fp32 rounded to 11-bit mantissa, full
PE rate at >=256 moving cols; only input-rounding error ~1e-4):
  kvT[kd|vd,lk]= Wkv.T @ xkvT   (kv projection FIRST so attention can start)
  RoPE in a halves-permuted head-dim basis (host permutes Wq/Wk columns):
  rot(x) = x*cos + Swap @ (x*sin_signed), Swap a 128x128 PE permutation.
  Attention per head-pair j (q proj for pair j+1 interleaved):
    S^T[lk,lq] = k-chunk.T @ qT  (two 1-bank matmuls into a 2-bank PSUM st)
    P = exp(S^T) [ACT, 2-bank supertile] * maskT [one DVE mult, partition-
    broadcast AP over the two banks]
    O_aug^T += V_aug-chunk.T @ P  (ones column -> row 64 = softmax denom)
    normalize: denom rows -> SBUF (DMA), reciprocal_approx_fast (DVE),
    gpsimd partition_broadcast, two DVE mults (no PE involvement).
  yT[f,lq] = Wo-chunks.T @ obig (+bo), Wo preloaded during attention.
"""

import ml_dtypes
import numpy as np

import concourse.bass as bass
import concourse.tile as tile
from concourse import bacc, mybir
from concourse import bass_utils
from concourse.bass import ts
from concourse.masks import make_identity

F32 = mybir.dt.float32
F32R = mybir.dt.float32r
FP16 = mybir.dt.float16

B, L, F, H, D = 2, 2048, 1024, 16, 64
LQ = 512            # query rows per core
LK = 2048           # kv rows (full)
NCORES = 8
PAIRS = H // 2      # head pairs (one qT partition block each)
FCH = F // 128      # f contraction chunks
KCH = LK // 128     # lk chunks
NL = LK // LQ       # kv column blocks

_CACHED = {}


def round_f32r(x: np.ndarray) -> np.ndarray:
    """Round-to-nearest-even fp32 -> fp32r (11-bit stored mantissa)."""
    u = np.ascontiguousarray(x, dtype=np.float32).view(np.uint32)
    lsb = (u >> np.uint32(12)) & np.uint32(1)
    u2 = (u + np.uint32(0x7FF) + lsb) & np.uint32(0xFFFFF000)
    return u2.view(np.float32)


def build_nc(debug=False):
    nc = bacc.Bacc("TRN2", target_bir_lowering=False, debug=False,
                   num_devices=NCORES)
    dt_in = [
        ("xq_t", [FCH, 128, LQ], F32R),        # [f, p, lq]
        ("xkv_t", [NL, FCH, 128, LQ], F32R),   # [l, f, p, lq]
        ("mask_t", [KCH, 128, LQ], FP16),      # [c, p, lq]
        ("wq", [FCH, 128, FCH, 128], F32R),    # [j, p, f, m]
        ("wkv", [128, FCH, 128], F32R),        # [p, f, m]
        ("wo", [FCH, 128, FCH, 128], F32R),    # [fb, p, j, m]
        ("bqbo", [128, 2 * FCH], F32),         # cols 0:8 bq-blocks, 8:16 bo
        ("bkv", [2 * D], F32),
        ("cosq", [128, LQ], F32),
        ("sinq", [128, LQ], F32),
        ("cksk", [D, 2 * LK], F32),            # [p, (cos|sin)*lk]
    ]
    t = {name: nc.dram_tensor(name, shape, dt, kind="ExternalInput")
         for name, shape, dt in dt_in}
    yT = nc.dram_tensor("yT", [F, LQ], F32, kind="ExternalOutput")
    dbg = {}
    if debug:
        for name, shape in [("d_qrot", [128, PAIRS, LQ]),
                            ("d_ktop", [128, LK]), ("d_kbot", [128, LK]),
                            ("d_vaug", [128, KCH, D + 1]),
                            ("d_obig", [128, PAIRS, LQ]),
                            ("d_oraw0", [D + 1, 2, LQ])]:
            dbg[name] = nc.dram_tensor(name, shape, F32, kind="ExternalOutput")
        dbg["d_pt0"] = nc.dram_tensor("d_pt0", [128, 2, LQ], FP16,
                                      kind="ExternalOutput")
        dbg["d_vaug16"] = nc.dram_tensor("d_vaug16", [128, KCH, D + 1], FP16,
                                         kind="ExternalOutput")
        dbg["d_rb0"] = nc.dram_tensor("d_rb0", [64, 2, LQ], F32,
                                      kind="ExternalOutput")
        dbg["d_rec0"] = nc.dram_tensor("d_rec0", [1, 2, LQ], F32,
                                       kind="ExternalOutput")

    with tile.TileContext(nc) as tc:
        with (
            tc.tile_pool(name="persist", bufs=1) as persist,
            tc.tile_pool(name="small", bufs=4) as small,
        ):
            # ---------- persistent SBUF ----------
            qrot = persist.tile([128, PAIRS, LQ], F32R)   # rotated qT
            ktop = persist.tile([128, LK], F32R)          # k in rows 0:64
            kbot = persist.tile([128, LK], F32R)          # k in rows 64:128
            vaug = persist.tile([128, KCH, D + 1], FP16)  # V chunks + ones
            obig = persist.tile([128, PAIRS, LQ], F32R)   # normalized O^T
            mt = persist.tile([128, KCH, LQ], FP16)       # maskT resident
            cq = persist.tile([128, LQ], F32)
            sq = persist.tile([128, LQ], F32)
            cksk = persist.tile([D, 2, LK], F32)
            wos = persist.tile([128, FCH, FCH, 128], F32R)  # Wo resident

            # scalar queue: small tables (after wkv, which phase A issues
            # first on this queue)
            nc.scalar.dma_start(cq, t["cosq"].ap())
            nc.scalar.dma_start(sq, t["sinq"].ap())
            nc.scalar.dma_start(cksk,
                                t["cksk"].ap().rearrange("p (a l) -> p a l",
                                                         a=2))
            bqbo = small.tile([128, 2 * FCH], F32, tag="bias")
            nc.scalar.dma_start(bqbo, t["bqbo"].ap())
            bq_sb = bqbo[:, 0:FCH]
            bo_sb = bqbo[:, FCH:2 * FCH]
            bkv_sb = small.tile([128, 1], F32, tag="bias2")
            nc.scalar.dma_start(bkv_sb, t["bkv"].ap().unsqueeze(1))
            # gpsimd queue: mask (one rearranged DMA) + wo (late)
            nc.gpsimd.dma_start(mt, t["mask_t"].ap().rearrange("c p l -> p c l"))

            idt = small.tile([128, 128], F32, tag="ident")
            make_identity(nc, idt)
            # halves-swap permutation matrix: M[p, p-xor-32-within-head] = 1
            swpf = small.tile([128, 128], F32, tag="swpf")
            nc.gpsimd.memset(swpf, 0.0)
            for o1, o2 in ((0, 32), (32, 0), (64, 96), (96, 64)):
                nc.gpsimd.affine_select(
                    out=swpf[o1:o1 + 32, o2:o2 + 32],
                    in_=swpf[o1:o1 + 32, o2:o2 + 32],
                    compare_op=mybir.AluOpType.not_equal, fill=1.0,
                    base=0, pattern=[[-1, 32]], channel_multiplier=1)
            swp = small.tile([128, 128], F32R, tag="swp")
            nc.vector.tensor_copy(swp, swpf)

            # ================= phase A: kv projection + rope-k ===========
            with (
                tc.tile_pool(name="xin", bufs=2) as xin,
                tc.tile_pool(name="wst", bufs=2) as wst,
                tc.tile_pool(name="kvraw", bufs=1) as kvp,
                tc.tile_pool(name="ktmp", bufs=1) as ktp,
                tc.tile_pool(name="pskv", bufs=2, space="PSUM") as pskv,
            ):
                wkv_sb = wst.tile([128, FCH, 128], F32R, tag="wkv")
                nc.scalar.dma_start(wkv_sb, t["wkv"].ap())
                kvraw = kvp.tile([128, LK], F32)
                for l in range(NL):
                    xkv = xin.tile([128, FCH, LQ], F32R, tag="x")
                    if l == 0:
                        for f in range(FCH):
                            nc.sync.dma_start(xkv[:, f, :],
                                              t["xkv_t"].ap()[l, f])
                    else:
                        nc.sync.dma_start(
                            xkv, t["xkv_t"].ap()[l].rearrange("f p l -> p f l"))
                    pkv = pskv.tile([128, LQ], F32, tag="acc")
                    for f in range(FCH):
                        nc.tensor.matmul(pkv, wkv_sb[:, f, :], xkv[:, f, :],
                                         start=(f == 0), stop=(f == FCH - 1))
                    nc.vector.tensor_scalar_add(kvraw[:, ts(l, LQ)], pkv,
                                                bkv_sb[:, 0:1])

                # ---- RoPE on k: matmul-swap; kbot copy via DMA ----
                ck = cksk[:, 0, :]
                sk = cksk[:, 1, :]
                tmk = ktp.tile([D, LK], F32R, tag="ksin")
                nc.vector.tensor_mul(tmk, kvraw[0:64], sk)
                kc = ktp.tile([D, LK], F32, tag="kcos")
                nc.vector.tensor_mul(kc, kvraw[0:64], ck)
                nc.vector.memset(ktop[64:128].bitcast(F32), 0.0)
                nc.vector.memset(kbot[0:64].bitcast(F32), 0.0)
                for l in range(NL):
                    pswk = pskv.tile([128, LQ], F32, tag="acc")
                    nc.tensor.matmul(pswk[0:64], swp[0:64, 0:64],
                                     tmk[:, ts(l, LQ)], start=True, stop=True)
                    nc.vector.tensor_add(ktop[0:64, ts(l, LQ)],
                                         kc[:, ts(l, LQ)], pswk[0:64])
                nc.gpsimd.dma_start(kbot[64:128], ktop[0:64])

                # ---- V_aug: transpose v chunks, append ones column ----
                nc.vector.memset(vaug[:, :, D:D + 1], 1.0)
                for c in range(KCH):
                    tp = pskv.tile([128, 512], F32, tag="acc")
                    nc.tensor.transpose(tp[:, 0:64], kvraw[64:128, ts(c, 128)],
                                        idt[64:128, 64:128])
                    nc.vector.tensor_copy(vaug[:, c, 0:D], tp[:, 0:64])

                # Wo preload on gpsimd queue (transfers during attention)
                nc.gpsimd.dma_start(wos, t["wo"].ap().rearrange(
                    "fb p j m -> p fb j m"))

            # ============ phase B/C: q proj (interleaved) + attention ====
            with (
                tc.tile_pool(name="xq", bufs=1) as xqp,
                tc.tile_pool(name="wst2", bufs=2) as wst2,
                tc.tile_pool(name="ropetmp", bufs=2) as rtp,
                tc.tile_pool(name="ptiles", bufs=3) as ptp,
                tc.tile_pool(name="norm", bufs=2) as nrm,
                tc.tile_pool(name="psq", bufs=2, space="PSUM") as psqp,
                tc.tile_pool(name="psst", bufs=2, space="PSUM") as psst,
                tc.tile_pool(name="oacc", bufs=1, space="PSUM") as oacc,
            ):
                xq = xqp.tile([128, FCH, LQ], F32R)
                nc.scalar.dma_start(
                    xq, t["xq_t"].ap().rearrange("f p l -> p f l"))

                def qproj(j):
                    wq_j = wst2.tile([128, FCH, 128], F32R, tag="w")
                    nc.gpsimd.dma_start(wq_j, t["wq"].ap()[j])
                    psq = psqp.tile([128, LQ], F32, tag="acc")
                    for f in range(FCH):
                        nc.tensor.matmul(psq, wq_j[:, f, :], xq[:, f, :],
                                         start=(f == 0), stop=(f == FCH - 1))
                    # tmq = (psq + bq)*sin ; qc = (psq + bq)*cos  (fused stt;
                    # must be DVE - gpsimd cannot read PSUM)
                    tmq = rtp.tile([128, LQ], F32R, tag="qsin")
                    nc.vector.scalar_tensor_tensor(
                        tmq, psq, bq_sb[:, j:j + 1], sq,
                        mybir.AluOpType.add, mybir.AluOpType.mult)
                    qc = rtp.tile([128, LQ], F32, tag="qcos")
                    nc.vector.scalar_tensor_tensor(
                        qc, psq, bq_sb[:, j:j + 1], cq,
                        mybir.AluOpType.add, mybir.AluOpType.mult)
                    psw = psqp.tile([128, LQ], F32, tag="acc")
                    nc.tensor.matmul(psw, swp, tmq, start=True, stop=True)
                    nc.vector.tensor_add(qrot[:, j, :], qc, psw)

                qproj(0)
                for j in range(PAIRS):
                    if j + 1 < PAIRS:
                        qproj(j + 1)
                    oab = oacc.tile([128, 2, LQ], F32, tag="oab")
                    for c in range(KCH):
                        st = psst.tile([128, 2, LQ], F32, tag="st")
                        nc.tensor.matmul(st[:, 0, :], ktop[:, ts(c, 128)],
                                         qrot[:, j, :], start=True, stop=True)
                        nc.tensor.matmul(st[:, 1, :], kbot[:, ts(c, 128)],
                                         qrot[:, j, :], start=True, stop=True)
                        pt = ptp.tile([128, 2, LQ], FP16, tag="p")
                        nc.scalar.activation(pt, st,
                                             mybir.ActivationFunctionType.Exp)
                        nc.vector.tensor_mul(
                            pt, pt,
                            mt[:, c:c + 1, :].broadcast_to([128, 2, LQ]))
                        if debug and j == 0 and c == 0:
                            nc.sync.dma_start(dbg["d_pt0"].ap(), pt)
                        nc.tensor.matmul(oab[0:D + 1, 0, :], vaug[:, c, :],
                                         pt[:, 0, :], start=(c == 0),
                                         stop=(c == KCH - 1))
                        nc.tensor.matmul(oab[0:D + 1, 1, :], vaug[:, c, :],
                                         pt[:, 1, :], start=(c == 0),
                                         stop=(c == KCH - 1))
                    # ---- normalization (no PE). One DVE copy stages O+denom
                    # to SBUF, freeing the PSUM accumulator for pair j+1;
                    # recip + partition-broadcast + muls then run off the
                    # critical path on the staged copy.
                    oraw = nrm.tile([D + 1, 2, LQ], F32, tag="oraw")
                    nc.vector.tensor_copy(oraw, oab[0:D + 1, :, :])
                    # denom row to a partition-0 tile (SBUF->SBUF DMA):
                    # custom-DVE ops ignore the AP partition offset, so
                    # reciprocal_approx_fast must read from partition 0.
                    den = nrm.tile([1, 2, LQ], F32, tag="den")
                    nc.gpsimd.dma_start(den, oraw[D:D + 1, :, :])
                    rec = nrm.tile([1, 2, LQ], F32, tag="rec")
                    nc.vector.reciprocal_approx_fast(rec, den)
                    rbA = nrm.tile([64, LQ], F32, tag="rbA")
                    rbB = nrm.tile([64, LQ], F32, tag="rbB")
                    nc.gpsimd.partition_broadcast(rbA, rec[0:1, 0, :])
                    nc.gpsimd.partition_broadcast(rbB, rec[0:1, 1, :])
                    nc.vector.tensor_mul(obig[0:64, j, :], oraw[0:64, 0, :],
                                         rbA)
                    osb = nrm.tile([64, LQ], F32R, tag="osb")
                    nc.vector.tensor_mul(osb, oraw[0:64, 1, :], rbB)
                    nc.gpsimd.dma_start(obig[64:128, j, :], osb)
                    if debug and j == 0:
                        nc.sync.dma_start(dbg["d_oraw0"].ap(), oraw)
                        nc.sync.dma_start(dbg["d_rb0"].ap()[:, 0, :], rbA)
                        nc.sync.dma_start(dbg["d_rb0"].ap()[:, 1, :], rbB)
                        nc.sync.dma_start(dbg["d_rec0"].ap(), rec)

            if debug:
                nc.sync.dma_start(dbg["d_qrot"].ap(), qrot.bitcast(F32))
                nc.sync.dma_start(dbg["d_ktop"].ap(), ktop.bitcast(F32))
                nc.sync.dma_start(dbg["d_kbot"].ap(), kbot.bitcast(F32))
                nc.sync.dma_start(dbg["d_vaug16"].ap(), vaug)
                nc.sync.dma_start(dbg["d_obig"].ap(), obig.bitcast(F32))

            # ================= phase D: output projection =================
            with (
                tc.tile_pool(name="yout", bufs=2) as yout,
                tc.tile_pool(name="psy", bufs=2, space="PSUM") as psyp,
            ):
                for fb in range(FCH):
                    psy = psyp.tile([128, LQ], F32, tag="acc")
                    for j in range(FCH):
                        nc.tensor.matmul(psy, wos[:, fb, j, :], obig[:, j, :],
                                         start=(j == 0), stop=(j == FCH - 1))
                    ysb = yout.tile([128, LQ], F32, tag="y")
                    nc.vector.tensor_scalar_add(ysb, psy, bo_sb[:, fb:fb + 1])
                    nc.sync.dma_start(yT.ap()[ts(fb, 128), :], ysb)

    nc.compile()
    return nc


def _tables():
    """RoPE tables in halves-permuted basis: rows i (even-half) hold +sin,
    rows 32+i (odd-half) hold -sin (for the tmp-then-swap formulation)."""
    inv_freq = 1.0 / (10000.0 ** (np.arange(0, D, 2, dtype=np.float64) / D))
    ang = np.outer(inv_freq, np.arange(L, dtype=np.float64))  # [32, L]
    cos = np.cos(ang).astype(np.float32)
    sin = np.sin(ang).astype(np.float32)
    cos64 = np.concatenate([cos, cos], axis=0)                # [64, L]
    sin_sgn = np.concatenate([sin, -sin], axis=0)             # [64, L]
    return cos64, sin_sgn


def _prep_weights(Wq, bq, Wk, bk, Wv, bv, Wo, bo):
    perm = np.concatenate([np.arange(0, D, 2), np.arange(1, D, 2)])
    WqP = np.asarray(Wq, dtype=np.float32)[:, :, perm].reshape(F, H * D)
    bqP = np.asarray(bq, dtype=np.float32)[:, perm].reshape(H * D)
    WkP = np.asarray(Wk, dtype=np.float32)[:, perm]
    bkP = np.asarray(bk, dtype=np.float32)[perm]
    Wkv = np.concatenate([WkP, np.asarray(Wv, dtype=np.float32)], axis=1)
    bkv = np.concatenate([bkP, np.asarray(bv, dtype=np.float32)])
    WoR = np.asarray(Wo, dtype=np.float32).reshape(H * D, F)
    bo_ = np.asarray(bo, dtype=np.float32)

    wq_pret = round_f32r(np.ascontiguousarray(
        WqP.reshape(FCH, 128, FCH, 128).transpose(2, 1, 0, 3)))
    wkv_pret = round_f32r(np.ascontiguousarray(
        Wkv.reshape(FCH, 128, 128).transpose(1, 0, 2)))
    wo_pret = round_f32r(np.ascontiguousarray(
        WoR.reshape(FCH, 128, FCH, 128).transpose(2, 1, 0, 3)))
    bqbo = np.ascontiguousarray(np.concatenate(
        [bqP.reshape(FCH, 128).T, bo_.reshape(FCH, 128).T], axis=1))
    return wq_pret, wkv_pret, wo_pret, bqbo, bkv


def kernel(inputs_q, inputs_kv, mask, Wq, bq, Wk, bk, Wv, bv, Wo, bo):
    if "nc" not in _CACHED:
        _CACHED["nc"] = build_nc()
    nc = _CACHED["nc"]

    wq_pret, wkv_pret, wo_pret, bqbo, bkv = _prep_weights(
        Wq, bq, Wk, bk, Wv, bv, Wo, bo)

    cos64, sin_sgn = _tables()
    scale = 1.0 / np.sqrt(np.float32(D))
    cksk = np.ascontiguousarray(
        np.concatenate([cos64, sin_sgn], axis=1))      # [64, 2*L] (L=LK)
    cosq_full = np.tile(cos64 * scale, (2, 1))         # [128, L]
    sinq_full = np.tile(sin_sgn * scale, (2, 1))

    xq = np.asarray(inputs_q, dtype=np.float32)
    xkv = np.asarray(inputs_kv, dtype=np.float32)
    mk = np.asarray(mask)

    in_maps = []
    for core in range(NCORES):
        b = core // 4
        qs = (core % 4) * LQ
        xq_t = round_f32r(np.ascontiguousarray(
            xq[b, qs:qs + LQ, :].T.reshape(FCH, 128, LQ)))
        xkv_t = round_f32r(np.ascontiguousarray(
            xkv[b].T.reshape(FCH, 128, NL, LQ).transpose(2, 0, 1, 3)))
        mask_t = np.ascontiguousarray(
            mk[b, 0, qs:qs + LQ, :].T.reshape(KCH, 128, LQ)
            .astype(np.float16))
        in_maps.append({
            "xq_t": xq_t,
            "xkv_t": xkv_t,
            "mask_t": mask_t,
            "wq": wq_pret,
            "wkv": wkv_pret,
            "wo": wo_pret,
            "bqbo": bqbo,
            "bkv": bkv,
            "cosq": np.ascontiguousarray(cosq_full[:, qs:qs + LQ]),
            "sinq": np.ascontiguousarray(sinq_full[:, qs:qs + LQ]),
            "cksk": cksk,
        })

    res = bass_utils.run_bass_kernel_spmd(nc, in_maps,
                                          core_ids=list(range(NCORES)))
    _CACHED["last_results"] = res
    _CACHED["last_maps"] = in_maps

    out = np.empty((B, L, F), dtype=np.float32)
    for core in range(NCORES):
        b = core // 4
        qs = (core % 4) * LQ
        out[b, qs:qs + LQ, :] = res.results[core]["yT"].T
    return out


# revision 30
# speedup vs baseline: 1.0975x; 1.0975x over previous
"""MQA attention (B=2, Lq=Lkv=2048, F=1024, H=16, D=64) on 8 TRN2 cores.

Sharding: core = (batch, query-block-of-512). Each core computes its full
output rows (all 16 heads + output projection) -> no collectives; host
concatenates per-core yT slabs.

Per-core dataflow (matmuls in f32r = fp32 rounded to 11-bit mantissa, full
PE rate at >=256 moving cols; only input-rounding error ~1e-4):
  kvT[kd|vd,lk]= Wkv.T @ xkvT   (kv projection FIRST so attention can start)
  RoPE in a halves-permuted head-dim basis (host permutes Wq/Wk columns):
  rot(x) = x*cos + Swap @ (x*sin_signed), Swap a 128x128 PE permutation.
  Attention per head-pair j (q proj for pair j+1 interleaved):
    S^T[lk,lq] = k-chunk.T @ qT  (two 1-bank matmuls into a 2-bank PSUM st)
    P = exp(S^T) [ACT, 2-bank supertile] * maskT [one DVE mult, partition-
    broadcast AP over the two banks]
    O_aug^T += V_aug-chunk.T @ P  (ones column -> row 64 = softmax denom)
    normalize: denom rows -> SBUF (DMA), reciprocal_approx_fast (DVE),
    gpsimd partition_broadcast, two DVE mults (no PE involvement).
  yT[f,lq] = Wo-chunks.T @ obig (+bo), Wo preloaded during attention.
"""

import ml_dtypes
import numpy as np

import concourse.bass as bass
import concourse.tile as tile
from concourse import bacc, mybir
from concourse import bass_utils
from concourse.bass import ts
from concourse.masks import make_identity

F32 = mybir.dt.float32
F32R = mybir.dt.float32r
FP16 = mybir.dt.float16
BF16 = mybir.dt.bfloat16

B, L, F, H, D = 2, 2048, 1024, 16, 64
LQ = 512            # query rows per core
LK = 2048           # kv rows (full)
NCORES = 8
PAIRS = H // 2      # head pairs (one qT partition block each)
FCH = F // 128      # f contraction chunks
KCH = LK // 128     # lk chunks
NL = LK // LQ       # kv column blocks

_CACHED = {}


def round_f32r(x: np.ndarray) -> np.ndarray:
    """Round-to-nearest-even fp32 -> fp32r (11-bit stored mantissa)."""
    u = np.ascontiguousarray(x, dtype=np.float32).view(np.uint32)
    lsb = (u >> np.uint32(12)) & np.uint32(1)
    u2 = (u + np.uint32(0x7FF) + lsb) & np.uint32(0xFFFFF000)
    return u2.view(np.float32)


def build_nc(debug=False):
    nc = bacc.Bacc("TRN2", target_bir_lowering=False, debug=False,
                   num_devices=NCORES)
    dt_in = [
        ("xq_t", [FCH, 128, LQ], BF16),        # [f, p, lq]
        ("xkv_t", [NL, FCH, 128, LQ], BF16),   # [l, f, p, lq]
        ("mask_t", [KCH, 128, LQ], FP16),      # [c, p, lq]
        ("wq", [FCH, 128, FCH, 128], BF16),    # [j, p, f, m]
        ("wkv", [128, FCH, 128], BF16),        # [p, f, m]
        ("wo", [FCH, 128, FCH, 128], BF16),    # [fb, p, j, m]
        ("bqbo", [128, 2 * FCH], F32),         # cols 0:8 bq-blocks, 8:16 bo
        ("bkv", [2 * D], F32),
        ("cosq", [128, LQ], F32),
        ("sinq", [128, LQ], F32),
        ("cksk", [D, 2 * LK], F32),            # [p, (cos|sin)*lk]
    ]
    t = {name: nc.dram_tensor(name, shape, dt, kind="ExternalInput")
         for name, shape, dt in dt_in}
    yT = nc.dram_tensor("yT", [F, LQ], F32, kind="ExternalOutput")
    dbg = {}
    if debug:
        for name, shape in [("d_qrot", [128, PAIRS, LQ]),
                            ("d_ktop", [128, LK]), ("d_kbot", [128, LK]),
                            ("d_vaug", [128, KCH, D + 1]),
                            ("d_oraw0", [D + 1, 2, LQ])]:
            dbg[name] = nc.dram_tensor(name, shape, F32, kind="ExternalOutput")
        dbg["d_pt0"] = nc.dram_tensor("d_pt0", [128, 2, LQ], FP16,
                                      kind="ExternalOutput")
        dbg["d_vaug16"] = nc.dram_tensor("d_vaug16", [128, KCH, D + 1], FP16,
                                         kind="ExternalOutput")
        dbg["d_rb0"] = nc.dram_tensor("d_rb0", [64, 2, LQ], F32,
                                      kind="ExternalOutput")
        dbg["d_rec0"] = nc.dram_tensor("d_rec0", [1, 2, LQ], F32,
                                       kind="ExternalOutput")
        dbg["d_obig"] = nc.dram_tensor("d_obig", [128, PAIRS, LQ], BF16,
                                       kind="ExternalOutput")

    with tile.TileContext(nc) as tc:
        with (
            tc.tile_pool(name="persist", bufs=1) as persist,
            tc.tile_pool(name="small", bufs=4) as small,
        ):
            # ---------- persistent SBUF ----------
            qrot = persist.tile([128, PAIRS, LQ], F32R)   # rotated qT
            ktop = persist.tile([128, LK], F32R)          # k in rows 0:64
            kbot = persist.tile([128, LK], F32R)          # k in rows 64:128
            vaug = persist.tile([128, KCH, D + 1], FP16)  # V chunks + ones
            obig = persist.tile([128, PAIRS, LQ], BF16)   # normalized O^T
            mt = persist.tile([128, KCH, LQ], FP16)       # maskT resident
            cq = persist.tile([128, LQ], F32)
            sq = persist.tile([128, LQ], F32)
            cksk = persist.tile([D, 2, LK], F32)
            wos = persist.tile([128, FCH, FCH, 128], BF16)  # Wo resident
            wkv_sb = persist.tile([128, FCH, 128], BF16)

            # scalar queue priority order: wkv (first kv matmul needs it),
            # then the small tables, then xq (issued in phase B).
            nc.scalar.dma_start(wkv_sb, t["wkv"].ap())
            nc.scalar.dma_start(cq, t["cosq"].ap())
            nc.scalar.dma_start(sq, t["sinq"].ap())
            nc.scalar.dma_start(cksk,
                                t["cksk"].ap().rearrange("p (a l) -> p a l",
                                                         a=2))
            bqbo = small.tile([128, 2 * FCH], F32, tag="bias")
            nc.scalar.dma_start(bqbo, t["bqbo"].ap())
            bq_sb = bqbo[:, 0:FCH]
            bo_sb = bqbo[:, FCH:2 * FCH]
            bkv_sb = small.tile([128, 1], F32, tag="bias2")
            nc.scalar.dma_start(bkv_sb, t["bkv"].ap().unsqueeze(1))
            # gpsimd queue: mask (one rearranged DMA) + wo (late)
            nc.gpsimd.dma_start(mt, t["mask_t"].ap().rearrange("c p l -> p c l"))

            idt = small.tile([128, 128], F32, tag="ident")
            make_identity(nc, idt)
            # halves-swap permutation matrix: M[p, p-xor-32-within-head] = 1
            swpf = small.tile([128, 128], F32, tag="swpf")
            nc.gpsimd.memset(swpf, 0.0)
            for o1, o2 in ((0, 32), (32, 0), (64, 96), (96, 64)):
                nc.gpsimd.affine_select(
                    out=swpf[o1:o1 + 32, o2:o2 + 32],
                    in_=swpf[o1:o1 + 32, o2:o2 + 32],
                    compare_op=mybir.AluOpType.not_equal, fill=1.0,
                    base=0, pattern=[[-1, 32]], channel_multiplier=1)
            swp = small.tile([128, 128], F32R, tag="swp")
            nc.vector.tensor_copy(swp, swpf)

            # ================= phase A: kv projection + rope-k ===========
            with (
                tc.tile_pool(name="xin", bufs=2) as xin,
                tc.tile_pool(name="kvraw", bufs=1) as kvp,
                tc.tile_pool(name="ktmp", bufs=1) as ktp,
                tc.tile_pool(name="pskv", bufs=2, space="PSUM") as pskv,
            ):
                kvraw = kvp.tile([128, LK], F32)
                for l in range(NL):
                    xkv = xin.tile([128, FCH, LQ], BF16, tag="x")
                    if l == 0:
                        for f in range(FCH):
                            nc.sync.dma_start(xkv[:, f, :],
                                              t["xkv_t"].ap()[l, f])
                    else:
                        nc.sync.dma_start(
                            xkv, t["xkv_t"].ap()[l].rearrange("f p l -> p f l"))
                    pkv = pskv.tile([128, LQ], F32, tag="acc")
                    for f in range(FCH):
                        nc.tensor.matmul(pkv, wkv_sb[:, f, :], xkv[:, f, :],
                                         start=(f == 0), stop=(f == FCH - 1))
                    nc.vector.tensor_scalar_add(kvraw[:, ts(l, LQ)], pkv,
                                                bkv_sb[:, 0:1])

                # ---- RoPE on k: matmul-swap; kbot copy via DMA ----
                ck = cksk[:, 0, :]
                sk = cksk[:, 1, :]
                tmk = ktp.tile([D, LK], F32R, tag="ksin")
                nc.vector.tensor_mul(tmk, kvraw[0:64], sk)
                kc = ktp.tile([D, LK], F32, tag="kcos")
                nc.vector.tensor_mul(kc, kvraw[0:64], ck)
                nc.vector.memset(ktop[64:128].bitcast(F32), 0.0)
                nc.vector.memset(kbot[0:64].bitcast(F32), 0.0)
                for l in range(NL):
                    pswk = pskv.tile([128, LQ], F32, tag="acc")
                    nc.tensor.matmul(pswk[0:64], swp[0:64, 0:64],
                                     tmk[:, ts(l, LQ)], start=True, stop=True)
                    nc.vector.tensor_add(ktop[0:64, ts(l, LQ)],
                                         kc[:, ts(l, LQ)], pswk[0:64])
                nc.gpsimd.dma_start(kbot[64:128], ktop[0:64])

                # ---- V_aug: transpose v chunks, append ones column ----
                nc.vector.memset(vaug[:, :, D:D + 1], 1.0)
                for c in range(KCH):
                    tp = pskv.tile([128, 512], F32, tag="acc")
                    nc.tensor.transpose(tp[:, 0:64], kvraw[64:128, ts(c, 128)],
                                        idt[64:128, 64:128])
                    nc.vector.tensor_copy(vaug[:, c, 0:D], tp[:, 0:64])

            # ============ phase B/C: q proj (interleaved) + attention ====
            with (
                tc.tile_pool(name="xq", bufs=1) as xqp,
                tc.tile_pool(name="wst2", bufs=2) as wst2,
                tc.tile_pool(name="ropetmp", bufs=2) as rtp,
                tc.tile_pool(name="ptiles", bufs=3) as ptp,
                tc.tile_pool(name="norm", bufs=2) as nrm,
                tc.tile_pool(name="psq", bufs=2, space="PSUM") as psqp,
                tc.tile_pool(name="psst", bufs=2, space="PSUM") as psst,
                tc.tile_pool(name="oacc", bufs=1, space="PSUM") as oacc,
            ):
                xq = xqp.tile([128, FCH, LQ], BF16)
                nc.scalar.dma_start(
                    xq, t["xq_t"].ap().rearrange("f p l -> p f l"))

                def qproj(j):
                    wq_j = wst2.tile([128, FCH, 128], BF16, tag="w")
                    nc.gpsimd.dma_start(wq_j, t["wq"].ap()[j])
                    psq = psqp.tile([128, LQ], F32, tag="acc")
                    for f in range(FCH):
                        nc.tensor.matmul(psq, wq_j[:, f, :], xq[:, f, :],
                                         start=(f == 0), stop=(f == FCH - 1))
                    # tmq = (psq + bq)*sin ; qc = (psq + bq)*cos  (fused stt;
                    # must be DVE - gpsimd cannot read PSUM)
                    tmq = rtp.tile([128, LQ], F32R, tag="qsin")
                    nc.vector.scalar_tensor_tensor(
                        tmq, psq, bq_sb[:, j:j + 1], sq,
                        mybir.AluOpType.add, mybir.AluOpType.mult)
                    qc = rtp.tile([128, LQ], F32, tag="qcos")
                    nc.vector.scalar_tensor_tensor(
                        qc, psq, bq_sb[:, j:j + 1], cq,
                        mybir.AluOpType.add, mybir.AluOpType.mult)
                    psw = psqp.tile([128, LQ], F32, tag="acc")
                    nc.tensor.matmul(psw, swp, tmq, start=True, stop=True)
                    nc.vector.tensor_add(qrot[:, j, :], qc, psw)

                def attn_o(oab, c, pt):
                    nc.tensor.matmul(oab[0:D + 1, 0, :], vaug[:, c, :],
                                     pt[:, 0, :], start=(c == 0),
                                     stop=(c == KCH - 1))
                    nc.tensor.matmul(oab[0:D + 1, 1, :], vaug[:, c, :],
                                     pt[:, 1, :], start=(c == 0),
                                     stop=(c == KCH - 1))

                qproj(0)
                for j in range(PAIRS):
                    if j + 1 < PAIRS:
                        qproj(j + 1)
                    if j == 1:
                        # Wo preload on the now-idle sync queue; streams
                        # during attention, needed only in phase D.
                        nc.sync.dma_start(wos, t["wo"].ap().rearrange(
                            "fb p j m -> p fb j m"))
                    oab = oacc.tile([128, 2, LQ], F32, tag="oab")
                    prev = None
                    for c in range(KCH):
                        st = psst.tile([128, 2, LQ], F32, tag="st")
                        nc.tensor.matmul(st[:, 0, :], ktop[:, ts(c, 128)],
                                         qrot[:, j, :], start=True, stop=True)
                        nc.tensor.matmul(st[:, 1, :], kbot[:, ts(c, 128)],
                                         qrot[:, j, :], start=True, stop=True)
                        pt = ptp.tile([128, 2, LQ], FP16, tag="p")
                        nc.scalar.activation(pt, st,
                                             mybir.ActivationFunctionType.Exp)
                        nc.vector.tensor_mul(
                            pt, pt,
                            mt[:, c:c + 1, :].broadcast_to([128, 2, LQ]))
                        if debug and j == 0 and c == 0:
                            nc.sync.dma_start(dbg["d_pt0"].ap(), pt)
                        # O matmuls run one chunk behind so they never make
                        # the PE wait on exp+mask of the current chunk.
                        if prev is not None:
                            attn_o(oab, c - 1, prev)
                        prev = pt
                    attn_o(oab, KCH - 1, prev)
                    # ---- normalization (no PE). One DVE copy stages O+denom
                    # to SBUF, freeing the PSUM accumulator for pair j+1;
                    # recip + partition-broadcast + muls then run off the
                    # critical path on the staged copy.
                    oraw = nrm.tile([D + 1, 2, LQ], F32, tag="oraw")
                    nc.vector.tensor_copy(oraw, oab[0:D + 1, :, :])
                    # denom row to a partition-0 tile (SBUF->SBUF DMA):
                    # custom-DVE ops ignore the AP partition offset, so
                    # reciprocal_approx_fast must read from partition 0.
                    den = nrm.tile([1, 2, LQ], F32, tag="den")
                    nc.gpsimd.dma_start(den, oraw[D:D + 1, :, :])
                    rec = nrm.tile([1, 2, LQ], F32, tag="rec")
                    nc.vector.reciprocal_approx_fast(rec, den)
                    rbA = nrm.tile([64, LQ], F32, tag="rbA")
                    rbB = nrm.tile([64, LQ], F32, tag="rbB")
                    nc.gpsimd.partition_broadcast(rbA, rec[0:1, 0, :])
                    nc.gpsimd.partition_broadcast(rbB, rec[0:1, 1, :])
                    nc.vector.tensor_mul(obig[0:64, j, :], oraw[0:64, 0, :],
                                         rbA)
                    osb = nrm.tile([64, LQ], BF16, tag="osb")
                    nc.vector.tensor_mul(osb, oraw[0:64, 1, :], rbB)
                    nc.gpsimd.dma_start(obig[64:128, j, :], osb)
                    if debug and j == 0:
                        nc.sync.dma_start(dbg["d_oraw0"].ap(), oraw)
                        nc.sync.dma_start(dbg["d_rb0"].ap()[:, 0, :], rbA)
                        nc.sync.dma_start(dbg["d_rb0"].ap()[:, 1, :], rbB)
                        nc.sync.dma_start(dbg["d_rec0"].ap(), rec)

            if debug:
                nc.sync.dma_start(dbg["d_qrot"].ap(), qrot.bitcast(F32))
                nc.sync.dma_start(dbg["d_ktop"].ap(), ktop.bitcast(F32))
                nc.sync.dma_start(dbg["d_kbot"].ap(), kbot.bitcast(F32))
                nc.sync.dma_start(dbg["d_vaug16"].ap(), vaug)
                nc.sync.dma_start(dbg["d_obig"].ap(), obig)

            # ================= phase D: output projection =================
            with (
                tc.tile_pool(name="yout", bufs=2) as yout,
                tc.tile_pool(name="psy", bufs=2, space="PSUM") as psyp,
            ):
                for fb in range(FCH):
                    psy = psyp.tile([128, LQ], F32, tag="acc")
                    for j in range(FCH):
                        nc.tensor.matmul(psy, wos[:, fb, j, :], obig[:, j, :],
                                         start=(j == 0), stop=(j == FCH - 1))
                    ysb = yout.tile([128, LQ], F32, tag="y")
                    nc.vector.tensor_scalar_add(ysb, psy, bo_sb[:, fb:fb + 1])
                    nc.sync.dma_start(yT.ap()[ts(fb, 128), :], ysb)

    nc.compile()
    return nc


def _tables():
    """RoPE tables in halves-permuted basis: rows i (even-half) hold +sin,
    rows 32+i (odd-half) hold -sin (for the tmp-then-swap formulation)."""
    inv_freq = 1.0 / (10000.0 ** (np.arange(0, D, 2, dtype=np.float64) / D))
    ang = np.outer(inv_freq, np.arange(L, dtype=np.float64))  # [32, L]
    cos = np.cos(ang).astype(np.float32)
    sin = np.sin(ang).astype(np.float32)
    cos64 = np.concatenate([cos, cos], axis=0)                # [64, L]
    sin_sgn = np.concatenate([sin, -sin], axis=0)             # [64, L]
    return cos64, sin_sgn


def _prep_weights(Wq, bq, Wk, bk, Wv, bv, Wo, bo):
    perm = np.concatenate([np.arange(0, D, 2), np.arange(1, D, 2)])
    WqP = np.asarray(Wq, dtype=np.float32)[:, :, perm].reshape(F, H * D)
    bqP = np.asarray(bq, dtype=np.float32)[:, perm].reshape(H * D)
    WkP = np.asarray(Wk, dtype=np.float32)[:, perm]
    bkP = np.asarray(bk, dtype=np.float32)[perm]
    Wkv = np.concatenate([WkP, np.asarray(Wv, dtype=np.float32)], axis=1)
    bkv = np.concatenate([bkP, np.asarray(bv, dtype=np.float32)])
    WoR = np.asarray(Wo, dtype=np.float32).reshape(H * D, F)
    bo_ = np.asarray(bo, dtype=np.float32)

    wq_pret = np.ascontiguousarray(
        WqP.reshape(FCH, 128, FCH, 128).transpose(2, 1, 0, 3)).astype(
        ml_dtypes.bfloat16)
    wkv_pret = np.ascontiguousarray(
        Wkv.reshape(FCH, 128, 128).transpose(1, 0, 2)).astype(
        ml_dtypes.bfloat16)
    wo_pret = np.ascontiguousarray(
        WoR.reshape(FCH, 128, FCH, 128).transpose(2, 1, 0, 3)).astype(
        ml_dtypes.bfloat16)
    bqbo = np.ascontiguousarray(np.concatenate(
        [bqP.reshape(FCH, 128).T, bo_.reshape(FCH, 128).T], axis=1))
    return wq_pret, wkv_pret, wo_pret, bqbo, bkv


def kernel(inputs_q, inputs_kv, mask, Wq, bq, Wk, bk, Wv, bv, Wo, bo):
    if "nc" not in _CACHED:
        _CACHED["nc"] = build_nc()
    nc = _CACHED["nc"]

    wq_pret, wkv_pret, wo_pret, bqbo, bkv = _prep_weights(
        Wq, bq, Wk, bk, Wv, bv, Wo, bo)

    cos64, sin_sgn = _tables()
    scale = 1.0 / np.sqrt(np.float32(D))
    cksk = np.ascontiguousarray(
        np.concatenate([cos64, sin_sgn], axis=1))      # [64, 2*L] (L=LK)
    cosq_full = np.tile(cos64 * scale, (2, 1))         # [128, L]
    sinq_full = np.tile(sin_sgn * scale, (2, 1))

    xq = np.asarray(inputs_q, dtype=np.float32)
    xkv = np.asarray(inputs_kv, dtype=np.float32)
    mk = np.asarray(mask)

    in_maps = []
    for core in range(NCORES):
        b = core // 4
        qs = (core % 4) * LQ
        xq_t = np.ascontiguousarray(
            xq[b, qs:qs + LQ, :].T.reshape(FCH, 128, LQ)).astype(
            ml_dtypes.bfloat16)
        xkv_t = np.ascontiguousarray(
            xkv[b].T.reshape(FCH, 128, NL, LQ).transpose(2, 0, 1, 3)).astype(
            ml_dtypes.bfloat16)
        mask_t = np.ascontiguousarray(
            mk[b, 0, qs:qs + LQ, :].T.reshape(KCH, 128, LQ)
            .astype(np.float16))
        in_maps.append({
            "xq_t": xq_t,
            "xkv_t": xkv_t,
            "mask_t": mask_t,
            "wq": wq_pret,
            "wkv": wkv_pret,
            "wo": wo_pret,
            "bqbo": bqbo,
            "bkv": bkv,
            "cosq": np.ascontiguousarray(cosq_full[:, qs:qs + LQ]),
            "sinq": np.ascontiguousarray(sinq_full[:, qs:qs + LQ]),
            "cksk": cksk,
        })

    res = bass_utils.run_bass_kernel_spmd(nc, in_maps,
                                          core_ids=list(range(NCORES)))
    _CACHED["last_results"] = res
    _CACHED["last_maps"] = in_maps

    out = np.empty((B, L, F), dtype=np.float32)
    for core in range(NCORES):
        b = core // 4
        qs = (core % 4) * LQ
        out[b, qs:qs + LQ, :] = res.results[core]["yT"].T
    return out


# revision 38
# speedup vs baseline: 1.0978x; 1.0002x over previous
"""MQA attention (B=2, Lq=Lkv=2048, F=1024, H=16, D=64) on 8 TRN2 cores.

Sharding: core = (batch, query-block-of-512). Each core computes its full
output rows (all 16 heads + output projection) -> no collectives; host
concatenates per-core yT slabs.

Per-core dataflow (matmuls in f32r = fp32 rounded to 11-bit mantissa, full
PE rate at >=256 moving cols; only input-rounding error ~1e-4):
  kvT[kd|vd,lk]= Wkv.T @ xkvT   (kv projection FIRST so attention can start)
  RoPE in a halves-permuted head-dim basis (host permutes Wq/Wk columns):
  rot(x) = x*cos + Swap @ (x*sin_signed), Swap a 128x128 PE permutation.
  Attention per head-pair j (q proj for pair j+1 interleaved):
    S^T[lk,lq] = k-chunk.T @ qT  (two 1-bank matmuls into a 2-bank PSUM st)
    P = exp(S^T) [ACT, 2-bank supertile] * maskT [one DVE mult, partition-
    broadcast AP over the two banks]
    O_aug^T += V_aug-chunk.T @ P  (ones column -> row 64 = softmax denom)
    normalize: denom rows -> SBUF (DMA), reciprocal_approx_fast (DVE),
    gpsimd partition_broadcast, two DVE mults (no PE involvement).
  yT[f,lq] = Wo-chunks.T @ obig (+bo), Wo preloaded during attention.
"""

import ml_dtypes
import numpy as np

import concourse.bass as bass
import concourse.tile as tile
from concourse import bacc, mybir
from concourse import bass_utils
from concourse.bass import ts
from concourse.masks import make_identity

F32 = mybir.dt.float32
F32R = mybir.dt.float32r
FP16 = mybir.dt.float16
BF16 = mybir.dt.bfloat16

B, L, F, H, D = 2, 2048, 1024, 16, 64
LQ = 512            # query rows per core
LK = 2048           # kv rows (full)
NCORES = 8
PAIRS = H // 2      # head pairs (one qT partition block each)
FCH = F // 128      # f contraction chunks
KCH = LK // 128     # lk chunks
NL = LK // LQ       # kv column blocks

_CACHED = {}


def round_f32r(x: np.ndarray) -> np.ndarray:
    """Round-to-nearest-even fp32 -> fp32r (11-bit stored mantissa)."""
    u = np.ascontiguousarray(x, dtype=np.float32).view(np.uint32)
    lsb = (u >> np.uint32(12)) & np.uint32(1)
    u2 = (u + np.uint32(0x7FF) + lsb) & np.uint32(0xFFFFF000)
    return u2.view(np.float32)


def build_nc(debug=False):
    nc = bacc.Bacc("TRN2", target_bir_lowering=False, debug=False,
                   num_devices=NCORES)
    dt_in = [
        ("xq_t", [FCH, 128, LQ], BF16),        # [f, p, lq]
        ("xkv_t", [NL, FCH, 128, LQ], BF16),   # [l, f, p, lq]
        ("mask_t", [KCH, 128, LQ], FP16),      # [c, p, lq]
        ("wq", [FCH, 128, FCH, 128], BF16),    # [j, p, f, m]
        ("wkv", [128, FCH, 128], BF16),        # [p, f, m]
        ("wo", [FCH, 128, FCH, 128], BF16),    # [fb, p, j, m]
        ("bqbo", [128, 2 * FCH], F32),         # cols 0:8 bq-blocks, 8:16 bo
        ("bkv", [2 * D], F32),
        ("cosq", [128, LQ], F32),
        ("sinq", [128, LQ], F32),
        ("cksk", [D, 2 * LK], F32),            # [p, (cos|sin)*lk]
    ]
    t = {name: nc.dram_tensor(name, shape, dt, kind="ExternalInput")
         for name, shape, dt in dt_in}
    yT = nc.dram_tensor("yT", [F, LQ], F32, kind="ExternalOutput")
    dbg = {}
    if debug:
        for name, shape in [("d_qrot", [128, PAIRS, LQ]),
                            ("d_ktop", [128, LK]), ("d_kbot", [128, LK]),
                            ("d_vaug", [128, KCH, D + 1]),
                            ("d_oraw0", [D + 1, 2, LQ])]:
            dbg[name] = nc.dram_tensor(name, shape, F32, kind="ExternalOutput")
        dbg["d_pt0"] = nc.dram_tensor("d_pt0", [128, 2, LQ], FP16,
                                      kind="ExternalOutput")
        dbg["d_vaug16"] = nc.dram_tensor("d_vaug16", [128, KCH, D + 1], FP16,
                                         kind="ExternalOutput")
        dbg["d_rb0"] = nc.dram_tensor("d_rb0", [64, 2, LQ], F32,
                                      kind="ExternalOutput")
        dbg["d_rec0"] = nc.dram_tensor("d_rec0", [1, 2, LQ], F32,
                                       kind="ExternalOutput")
        dbg["d_obig"] = nc.dram_tensor("d_obig", [128, PAIRS, LQ], BF16,
                                       kind="ExternalOutput")

    with tile.TileContext(nc) as tc:
        with (
            tc.tile_pool(name="persist", bufs=1) as persist,
            tc.tile_pool(name="small", bufs=4) as small,
        ):
            # ---------- persistent SBUF ----------
            # per-pair tiles (vs one big tile) so the Tile dep-tracker sees
            # no false write-after-read hazards between pairs
            qrt = [persist.tile([128, LQ], F32R, tag=f"qr{j}",
                                name=f"qrt{j}")
                   for j in range(PAIRS)]                 # rotated qT
            obt = [persist.tile([128, LQ], BF16, tag=f"ob{j}",
                                name=f"obt{j}")
                   for j in range(PAIRS)]                 # normalized O^T
            ktop = persist.tile([128, LK], F32R)          # k in rows 0:64
            kbot = persist.tile([128, LK], F32R)          # k in rows 64:128
            vaug = persist.tile([128, KCH, D + 1], FP16)  # V chunks + ones
            mt = persist.tile([128, KCH, LQ], FP16)       # maskT resident
            cq = persist.tile([128, LQ], F32)
            sq = persist.tile([128, LQ], F32)
            cksk = persist.tile([D, 2, LK], F32)
            wos = persist.tile([128, FCH, FCH, 128], BF16)  # Wo resident
            wkv_sb = persist.tile([128, FCH, 128], BF16)

            # scalar queue priority order: wkv (first kv matmul needs it),
            # then the small tables, then xq (issued in phase B).
            nc.scalar.dma_start(wkv_sb, t["wkv"].ap())
            nc.scalar.dma_start(cq, t["cosq"].ap())
            nc.scalar.dma_start(sq, t["sinq"].ap())
            nc.scalar.dma_start(cksk,
                                t["cksk"].ap().rearrange("p (a l) -> p a l",
                                                         a=2))
            bqbo = small.tile([128, 2 * FCH], F32, tag="bias")
            nc.scalar.dma_start(bqbo, t["bqbo"].ap())
            bq_sb = bqbo[:, 0:FCH]
            bo_sb = bqbo[:, FCH:2 * FCH]
            bkv_sb = small.tile([128, 1], F32, tag="bias2")
            nc.scalar.dma_start(bkv_sb, t["bkv"].ap().unsqueeze(1))

            idt = small.tile([128, 128], F32, tag="ident")
            make_identity(nc, idt)
            # halves-swap permutation matrix: M[p, p-xor-32-within-head] = 1
            swpf = small.tile([128, 128], F32, tag="swpf")
            nc.gpsimd.memset(swpf, 0.0)
            for o1, o2 in ((0, 32), (32, 0), (64, 96), (96, 64)):
                nc.gpsimd.affine_select(
                    out=swpf[o1:o1 + 32, o2:o2 + 32],
                    in_=swpf[o1:o1 + 32, o2:o2 + 32],
                    compare_op=mybir.AluOpType.not_equal, fill=1.0,
                    base=0, pattern=[[-1, 32]], channel_multiplier=1)
            swp = small.tile([128, 128], F32R, tag="swp")
            nc.vector.tensor_copy(swp, swpf)

            # ================= phase A: kv projection + rope-k ===========
            with (
                tc.tile_pool(name="xin", bufs=2) as xin,
                tc.tile_pool(name="kvraw", bufs=1) as kvp,
                tc.tile_pool(name="ktmp", bufs=1) as ktp,
                tc.tile_pool(name="pskv", bufs=2, space="PSUM") as pskv,
            ):
                kvraw = kvp.tile([128, LK], F32)
                ck = cksk[:, 0, :]
                sk = cksk[:, 1, :]
                tmk = ktp.tile([D, LK], F32R, tag="ksin")
                kc = ktp.tile([D, LK], F32, tag="kcos")
                nc.vector.memset(ktop[64:128].bitcast(F32), 0.0)
                nc.vector.memset(kbot[0:64].bitcast(F32), 0.0)
                nc.vector.memset(vaug[:, :, D:D + 1], 1.0)
                # kv-proj, rope-k and V-transposes fully pipelined per block
                for l in range(NL):
                    xkv = xin.tile([128, FCH, LQ], BF16, tag="x")
                    if l == 0:
                        for f in range(FCH):
                            nc.sync.dma_start(xkv[:, f, :],
                                              t["xkv_t"].ap()[l, f])
                    else:
                        nc.sync.dma_start(
                            xkv, t["xkv_t"].ap()[l].rearrange("f p l -> p f l"))
                    pkv = pskv.tile([128, LQ], F32, tag="acc")
                    for f in range(FCH):
                        nc.tensor.matmul(pkv, wkv_sb[:, f, :], xkv[:, f, :],
                                         start=(f == 0), stop=(f == FCH - 1))
                    nc.vector.tensor_scalar_add(kvraw[:, ts(l, LQ)], pkv,
                                                bkv_sb[:, 0:1])
                    lb = ts(l, LQ)
                    nc.vector.tensor_mul(tmk[:, lb], kvraw[0:64, lb],
                                         sk[:, lb])
                    nc.vector.tensor_mul(kc[:, lb], kvraw[0:64, lb],
                                         ck[:, lb])
                    pswk = pskv.tile([128, LQ], F32, tag="acc")
                    nc.tensor.matmul(pswk[0:64], swp[0:64, 0:64],
                                     tmk[:, lb], start=True, stop=True)
                    nc.vector.tensor_add(ktop[0:64, lb],
                                         kc[:, lb], pswk[0:64])
                    for c in range(4 * l, 4 * l + 4):
                        tp = pskv.tile([128, 512], F32, tag="acc")
                        nc.tensor.transpose(tp[:, 0:64],
                                            kvraw[64:128, ts(c, 128)],
                                            idt[64:128, 64:128])
                        nc.vector.tensor_copy(vaug[:, c, 0:D], tp[:, 0:64])
                nc.scalar.dma_start(kbot[64:128], ktop[0:64])

            # ============ phase B/C: q proj (interleaved) + attention ====
            with (
                tc.tile_pool(name="xq", bufs=1) as xqp,
                tc.tile_pool(name="wst2", bufs=2) as wst2,
                tc.tile_pool(name="ropetmp", bufs=2) as rtp,
                tc.tile_pool(name="ptiles", bufs=3) as ptp,
                tc.tile_pool(name="norm", bufs=2) as nrm,
                tc.tile_pool(name="psq", bufs=2, space="PSUM") as psqp,
                tc.tile_pool(name="psst", bufs=2, space="PSUM") as psst,
                tc.tile_pool(name="oacc", bufs=1, space="PSUM") as oacc,
            ):
                xq = xqp.tile([128, FCH, LQ], BF16)
                nc.scalar.dma_start(
                    xq, t["xq_t"].ap().rearrange("f p l -> p f l"))

                def qproj(j):
                    wq_j = wst2.tile([128, FCH, 128], BF16, tag="w")
                    nc.gpsimd.dma_start(wq_j, t["wq"].ap()[j])
                    psq = psqp.tile([128, LQ], F32, tag="acc")
                    for f in range(FCH):
                        nc.tensor.matmul(psq, wq_j[:, f, :], xq[:, f, :],
                                         start=(f == 0), stop=(f == FCH - 1))
                    # tmq = (psq + bq)*sin ; qc = (psq + bq)*cos  (fused stt;
                    # must be DVE - gpsimd cannot read PSUM)
                    tmq = rtp.tile([128, LQ], F32R, tag="qsin")
                    nc.vector.scalar_tensor_tensor(
                        tmq, psq, bq_sb[:, j:j + 1], sq,
                        mybir.AluOpType.add, mybir.AluOpType.mult)
                    qc = rtp.tile([128, LQ], F32, tag="qcos")
                    nc.vector.scalar_tensor_tensor(
                        qc, psq, bq_sb[:, j:j + 1], cq,
                        mybir.AluOpType.add, mybir.AluOpType.mult)
                    psw = psqp.tile([128, LQ], F32, tag="acc")
                    nc.tensor.matmul(psw, swp, tmq, start=True, stop=True)
                    nc.vector.tensor_add(qrt[j], qc, psw)

                def attn_o(oab, c, pt):
                    nc.tensor.matmul(oab[0:D + 1, 0, :], vaug[:, c, :],
                                     pt[:, 0, :], start=(c == 0),
                                     stop=(c == KCH - 1))
                    nc.tensor.matmul(oab[0:D + 1, 1, :], vaug[:, c, :],
                                     pt[:, 1, :], start=(c == 0),
                                     stop=(c == KCH - 1))

                qproj(0)
                # mask DMA behind wq0 on the gpsimd queue: needed only when
                # the first exp output is masked (~30us in)
                nc.gpsimd.dma_start(
                    mt, t["mask_t"].ap().rearrange("c p l -> p c l"))
                for j in range(PAIRS):
                    if j + 1 < PAIRS:
                        qproj(j + 1)
                    if j == 3:
                        # Wo preload on the now-idle sync queue; streams
                        # during attention, needed only in phase D.
                        nc.sync.dma_start(wos, t["wo"].ap().rearrange(
                            "fb p j m -> p fb j m"))
                    oab = oacc.tile([128, 2, LQ], F32, tag="oab")
                    prev = None
                    for c in range(KCH):
                        st = psst.tile([128, 2, LQ], F32, tag="st")
                        nc.tensor.matmul(st[:, 0, :], ktop[:, ts(c, 128)],
                                         qrt[j], start=True, stop=True)
                        nc.tensor.matmul(st[:, 1, :], kbot[:, ts(c, 128)],
                                         qrt[j], start=True, stop=True)
                        pt = ptp.tile([128, 2, LQ], FP16, tag="p")
                        nc.scalar.activation(pt, st,
                                             mybir.ActivationFunctionType.Exp)
                        nc.vector.tensor_mul(
                            pt, pt,
                            mt[:, c:c + 1, :].broadcast_to([128, 2, LQ]))
                        if debug and j == 0 and c == 0:
                            nc.sync.dma_start(dbg["d_pt0"].ap(), pt)
                        # O matmuls run one chunk behind so they never make
                        # the PE wait on exp+mask of the current chunk.
                        if prev is not None:
                            attn_o(oab, c - 1, prev)
                        prev = pt
                    attn_o(oab, KCH - 1, prev)
                    # ---- normalization (no PE). One DVE copy stages O+denom
                    # to SBUF, freeing the PSUM accumulator for pair j+1;
                    # recip + partition-broadcast + muls then run off the
                    # critical path on the staged copy.
                    oraw = nrm.tile([D + 1, 2, LQ], F32, tag="oraw")
                    nc.vector.tensor_copy(oraw, oab[0:D + 1, :, :])
                    # denom row to a partition-0 tile (SBUF->SBUF DMA):
                    # custom-DVE ops ignore the AP partition offset, so
                    # reciprocal_approx_fast must read from partition 0.
                    den = nrm.tile([1, 2, LQ], F32, tag="den")
                    nc.gpsimd.dma_start(den, oraw[D:D + 1, :, :])
                    rec = nrm.tile([1, 2, LQ], F32, tag="rec")
                    nc.vector.reciprocal_approx_fast(rec, den)
                    rbA = nrm.tile([64, LQ], F32, tag="rbA")
                    rbB = nrm.tile([64, LQ], F32, tag="rbB")
                    nc.gpsimd.partition_broadcast(rbA, rec[0:1, 0, :])
                    nc.gpsimd.partition_broadcast(rbB, rec[0:1, 1, :])
                    nc.vector.tensor_mul(obt[j][0:64, :], oraw[0:64, 0, :],
                                         rbA)
                    osb = nrm.tile([64, LQ], BF16, tag="osb")
                    nc.vector.tensor_mul(osb, oraw[0:64, 1, :], rbB)
                    nc.gpsimd.dma_start(obt[j][64:128, :], osb)
                    if debug and j == 0:
                        nc.sync.dma_start(dbg["d_oraw0"].ap(), oraw)
                        nc.sync.dma_start(dbg["d_rb0"].ap()[:, 0, :], rbA)
                        nc.sync.dma_start(dbg["d_rb0"].ap()[:, 1, :], rbB)
                        nc.sync.dma_start(dbg["d_rec0"].ap(), rec)

            if debug:
                for j in range(PAIRS):
                    nc.sync.dma_start(dbg["d_qrot"].ap()[:, j, :],
                                      qrt[j].bitcast(F32))
                    nc.sync.dma_start(dbg["d_obig"].ap()[:, j, :], obt[j])
                nc.sync.dma_start(dbg["d_ktop"].ap(), ktop.bitcast(F32))
                nc.sync.dma_start(dbg["d_kbot"].ap(), kbot.bitcast(F32))
                nc.sync.dma_start(dbg["d_vaug16"].ap(), vaug)

            # ================= phase D: output projection =================
            # all j<=6 partial accumulations first (they only need the first
            # 7 pairs' obt, so they overlap the last pair's normalization);
            # then the j=7 finish + bias + store per f-block.
            with (
                tc.tile_pool(name="yout", bufs=2) as yout,
                tc.tile_pool(name="psy", bufs=1, space="PSUM") as psyp,
            ):
                psys = [psyp.tile([128, LQ], F32, tag=f"y{fb}",
                                  name=f"psy{fb}")
                        for fb in range(FCH)]
                for fb in range(FCH):
                    for j in range(FCH - 1):
                        nc.tensor.matmul(psys[fb], wos[:, fb, j, :], obt[j],
                                         start=(j == 0), stop=False)
                for fb in range(FCH):
                    nc.tensor.matmul(psys[fb], wos[:, fb, FCH - 1, :],
                                     obt[FCH - 1], start=False, stop=True)
                    ysb = yout.tile([128, LQ], F32, tag="y")
                    nc.vector.tensor_scalar_add(ysb, psys[fb],
                                                bo_sb[:, fb:fb + 1])
                    nc.sync.dma_start(yT.ap()[ts(fb, 128), :], ysb)

    nc.compile()
    return nc


def _tables():
    """RoPE tables in halves-permuted basis: rows i (even-half) hold +sin,
    rows 32+i (odd-half) hold -sin (for the tmp-then-swap formulation)."""
    inv_freq = 1.0 / (10000.0 ** (np.arange(0, D, 2, dtype=np.float64) / D))
    ang = np.outer(inv_freq, np.arange(L, dtype=np.float64))  # [32, L]
    cos = np.cos(ang).astype(np.float32)
    sin = np.sin(ang).astype(np.float32)
    cos64 = np.concatenate([cos, cos], axis=0)                # [64, L]
    sin_sgn = np.concatenate([sin, -sin], axis=0)             # [64, L]
    return cos64, sin_sgn


def _prep_weights(Wq, bq, Wk, bk, Wv, bv, Wo, bo):
    perm = np.concatenate([np.arange(0, D, 2), np.arange(1, D, 2)])
    WqP = np.asarray(Wq, dtype=np.float32)[:, :, perm].reshape(F, H * D)
    bqP = np.asarray(bq, dtype=np.float32)[:, perm].reshape(H * D)
    WkP = np.asarray(Wk, dtype=np.float32)[:, perm]
    bkP = np.asarray(bk, dtype=np.float32)[perm]
    Wkv = np.concatenate([WkP, np.asarray(Wv, dtype=np.float32)], axis=1)
    bkv = np.concatenate([bkP, np.asarray(bv, dtype=np.float32)])
    WoR = np.asarray(Wo, dtype=np.float32).reshape(H * D, F)
    bo_ = np.asarray(bo, dtype=np.float32)

    wq_pret = np.ascontiguousarray(
        WqP.reshape(FCH, 128, FCH, 128).transpose(2, 1, 0, 3)).astype(
        ml_dtypes.bfloat16)
    wkv_pret = np.ascontiguousarray(
        Wkv.reshape(FCH, 128, 128).transpose(1, 0, 2)).astype(
        ml_dtypes.bfloat16)
    wo_pret = np.ascontiguousarray(
        WoR.reshape(FCH, 128, FCH, 128).transpose(2, 1, 0, 3)).astype(
        ml_dtypes.bfloat16)
    bqbo = np.ascontiguousarray(np.concatenate(
        [bqP.reshape(FCH, 128).T, bo_.reshape(FCH, 128).T], axis=1))
    return wq_pret, wkv_pret, wo_pret, bqbo, bkv


def kernel(inputs_q, inputs_kv, mask, Wq, bq, Wk, bk, Wv, bv, Wo, bo):
    if "nc" not in _CACHED:
        _CACHED["nc"] = build_nc()
    nc = _CACHED["nc"]

    wq_pret, wkv_pret, wo_pret, bqbo, bkv = _prep_weights(
        Wq, bq, Wk, bk, Wv, bv, Wo, bo)

    cos64, sin_sgn = _tables()
    scale = 1.0 / np.sqrt(np.float32(D))
    cksk = np.ascontiguousarray(
        np.concatenate([cos64, sin_sgn], axis=1))      # [64, 2*L] (L=LK)
    cosq_full = np.tile(cos64 * scale, (2, 1))         # [128, L]
    sinq_full = np.tile(sin_sgn * scale, (2, 1))

    xq = np.asarray(inputs_q, dtype=np.float32)
    xkv = np.asarray(inputs_kv, dtype=np.float32)
    mk = np.asarray(mask)

    in_maps = []
    for core in range(NCORES):
        b = core // 4
        qs = (core % 4) * LQ
        xq_t = np.ascontiguousarray(
            xq[b, qs:qs + LQ, :].T.reshape(FCH, 128, LQ)).astype(
            ml_dtypes.bfloat16)
        xkv_t = np.ascontiguousarray(
            xkv[b].T.reshape(FCH, 128, NL, LQ).transpose(2, 0, 1, 3)).astype(
            ml_dtypes.bfloat16)
        mask_t = np.ascontiguousarray(
            mk[b, 0, qs:qs + LQ, :].T.reshape(KCH, 128, LQ)
            .astype(np.float16))
        in_maps.append({
            "xq_t": xq_t,
            "xkv_t": xkv_t,
            "mask_t": mask_t,
            "wq": wq_pret,
            "wkv": wkv_pret,
            "wo": wo_pret,
            "bqbo": bqbo,
            "bkv": bkv,
            "cosq": np.ascontiguousarray(cosq_full[:, qs:qs + LQ]),
            "sinq": np.ascontiguousarray(sinq_full[:, qs:qs + LQ]),
            "cksk": cksk,
        })

    res = bass_utils.run_bass_kernel_spmd(nc, in_maps,
                                          core_ids=list(range(NCORES)))
    _CACHED["last_results"] = res
    _CACHED["last_maps"] = in_maps

    out = np.empty((B, L, F), dtype=np.float32)
    for core in range(NCORES):
        b = core // 4
        qs = (core % 4) * LQ
        out[b, qs:qs + LQ, :] = res.results[core]["yT"].T
    return out


# revision 40
# speedup vs baseline: 1.1121x; 1.0130x over previous
"""MQA attention (B=2, Lq=Lkv=2048, F=1024, H=16, D=64) on 8 TRN2 cores.

Sharding: core = (batch, query-block-of-512). Each core computes its full
output rows (all 16 heads + output projection) -> no collectives; host
concatenates per-core yT slabs.

Per-core dataflow (matmuls in f32r = fp32 rounded to 11-bit mantissa, full
PE rate at >=256 moving cols; only input-rounding error ~1e-4):
  kvT[kd|vd,lk]= Wkv.T @ xkvT   (kv projection FIRST so attention can start)
  RoPE in a halves-permuted head-dim basis (host permutes Wq/Wk columns):
  rot(x) = x*cos + Swap @ (x*sin_signed), Swap a 128x128 PE permutation.
  Attention per head-pair j (q proj for pair j+1 interleaved):
    S^T[lk,lq] = k-chunk.T @ qT  (two 1-bank matmuls into a 2-bank PSUM st)
    P = exp(S^T) [ACT, 2-bank supertile] * maskT [one DVE mult, partition-
    broadcast AP over the two banks]
    O_aug^T += V_aug-chunk.T @ P  (ones column -> row 64 = softmax denom)
    normalize: denom rows -> SBUF (DMA), reciprocal_approx_fast (DVE),
    gpsimd partition_broadcast, two DVE mults (no PE involvement).
  yT[f,lq] = Wo-chunks.T @ obig (+bo), Wo preloaded during attention.
"""

import ml_dtypes
import numpy as np

import concourse.bass as bass
import concourse.tile as tile
from concourse import bacc, mybir
from concourse import bass_utils
from concourse.bass import ts
from concourse.masks import make_identity

F32 = mybir.dt.float32
F32R = mybir.dt.float32r
FP16 = mybir.dt.float16
BF16 = mybir.dt.bfloat16

B, L, F, H, D = 2, 2048, 1024, 16, 64
LQ = 512            # query rows per core
LK = 2048           # kv rows (full)
NCORES = 8
PAIRS = H // 2      # head pairs (one qT partition block each)
FCH = F // 128      # f contraction chunks
KCH = LK // 128     # lk chunks
NL = LK // LQ       # kv column blocks

_CACHED = {}


def round_f32r(x: np.ndarray) -> np.ndarray:
    """Round-to-nearest-even fp32 -> fp32r (11-bit stored mantissa)."""
    u = np.ascontiguousarray(x, dtype=np.float32).view(np.uint32)
    lsb = (u >> np.uint32(12)) & np.uint32(1)
    u2 = (u + np.uint32(0x7FF) + lsb) & np.uint32(0xFFFFF000)
    return u2.view(np.float32)


def build_nc(debug=False):
    nc = bacc.Bacc("TRN2", target_bir_lowering=False, debug=False,
                   num_devices=NCORES)
    dt_in = [
        ("xq_t", [FCH, 128, LQ], BF16),        # [f, p, lq]
        ("xkv_t", [NL, FCH, 128, LQ], BF16),   # [l, f, p, lq]
        ("mask_t", [KCH, 128, LQ], FP16),      # [c, p, lq]
        ("wq", [FCH, 128, FCH, 128], BF16),    # [j, p, f, m]
        ("wkv", [128, FCH, 128], BF16),        # [p, f, m]
        ("wo", [FCH, 128, FCH, 128], BF16),    # [fb, p, j, m]
        ("bqbo", [128, 2 * FCH], F32),         # cols 0:8 bq-blocks, 8:16 bo
        ("bkv", [2 * D], F32),
        ("cosq", [128, LQ], F32),
        ("sinq", [128, LQ], F32),
        ("cksk", [D, 2 * LK], F32),            # [p, (cos|sin)*lk]
    ]
    t = {name: nc.dram_tensor(name, shape, dt, kind="ExternalInput")
         for name, shape, dt in dt_in}
    yT = nc.dram_tensor("yT", [F, LQ], F32, kind="ExternalOutput")
    dbg = {}
    if debug:
        for name, shape in [("d_qrot", [128, PAIRS, LQ]),
                            ("d_ktop", [128, LK]), ("d_kbot", [128, LK]),
                            ("d_vaug", [128, KCH, D + 1]),
                            ("d_oraw0", [D + 1, 2, LQ])]:
            dbg[name] = nc.dram_tensor(name, shape, F32, kind="ExternalOutput")
        dbg["d_pt0"] = nc.dram_tensor("d_pt0", [128, 2, LQ], FP16,
                                      kind="ExternalOutput")
        dbg["d_vaug16"] = nc.dram_tensor("d_vaug16", [128, KCH, D + 1], FP16,
                                         kind="ExternalOutput")
        dbg["d_rb0"] = nc.dram_tensor("d_rb0", [64, 2, LQ], F32,
                                      kind="ExternalOutput")
        dbg["d_rec0"] = nc.dram_tensor("d_rec0", [1, 2, LQ], F32,
                                       kind="ExternalOutput")
        dbg["d_obig"] = nc.dram_tensor("d_obig", [128, PAIRS, LQ], BF16,
                                       kind="ExternalOutput")

    with tile.TileContext(nc) as tc:
        with (
            tc.tile_pool(name="persist", bufs=1) as persist,
            tc.tile_pool(name="small", bufs=4) as small,
        ):
            # ---------- persistent SBUF ----------
            # per-pair tiles (vs one big tile) so the Tile dep-tracker sees
            # no false write-after-read hazards between pairs
            qrt = [persist.tile([128, LQ], F32R, tag=f"qr{j}",
                                name=f"qrt{j}")
                   for j in range(PAIRS)]                 # rotated qT
            obt = [persist.tile([128, LQ], BF16, tag=f"ob{j}",
                                name=f"obt{j}")
                   for j in range(PAIRS)]                 # normalized O^T
            ktop = persist.tile([128, LK], F32R)          # k in rows 0:64
            kbot = persist.tile([128, LK], F32R)          # k in rows 64:128
            vaug = persist.tile([128, KCH, D + 1], FP16)  # V chunks + ones
            mt = persist.tile([128, KCH, LQ], FP16)       # maskT resident
            cq = persist.tile([128, LQ], F32)
            sq = persist.tile([128, LQ], F32)
            cksk = persist.tile([D, 2, LK], F32)
            wos = persist.tile([128, FCH, FCH, 128], BF16)  # Wo resident
            wkv_sb = persist.tile([128, FCH, 128], BF16)

            # scalar queue priority order: wkv (first kv matmul needs it),
            # then the small tables, then xq (issued in phase B).
            nc.scalar.dma_start(wkv_sb, t["wkv"].ap())
            nc.scalar.dma_start(cq, t["cosq"].ap())
            nc.scalar.dma_start(sq, t["sinq"].ap())
            nc.scalar.dma_start(cksk,
                                t["cksk"].ap().rearrange("p (a l) -> p a l",
                                                         a=2))
            bqbo = small.tile([128, 2 * FCH], F32, tag="bias")
            nc.scalar.dma_start(bqbo, t["bqbo"].ap())
            bq_sb = bqbo[:, 0:FCH]
            bo_sb = bqbo[:, FCH:2 * FCH]
            bkv_sb = small.tile([128, 1], F32, tag="bias2")
            nc.scalar.dma_start(bkv_sb, t["bkv"].ap().unsqueeze(1))

            idt = small.tile([128, 128], F32, tag="ident")
            make_identity(nc, idt)
            # halves-swap permutation matrix: M[p, p-xor-32-within-head] = 1
            swpf = small.tile([128, 128], F32, tag="swpf")
            nc.gpsimd.memset(swpf, 0.0)
            for o1, o2 in ((0, 32), (32, 0), (64, 96), (96, 64)):
                nc.gpsimd.affine_select(
                    out=swpf[o1:o1 + 32, o2:o2 + 32],
                    in_=swpf[o1:o1 + 32, o2:o2 + 32],
                    compare_op=mybir.AluOpType.not_equal, fill=1.0,
                    base=0, pattern=[[-1, 32]], channel_multiplier=1)
            swp = small.tile([128, 128], F32R, tag="swp")
            nc.vector.tensor_copy(swp, swpf)

            # ================= phase A: kv projection + rope-k ===========
            with (
                tc.tile_pool(name="xin", bufs=4) as xin,
                tc.tile_pool(name="kvraw", bufs=1) as kvp,
                tc.tile_pool(name="ktmp", bufs=1) as ktp,
                tc.tile_pool(name="pskv", bufs=2, space="PSUM") as pskv,
            ):
                kvraw = kvp.tile([128, LK], F32)
                ck = cksk[:, 0, :]
                sk = cksk[:, 1, :]
                tmk = ktp.tile([D, LK], F32R, tag="ksin")
                kc = ktp.tile([D, LK], F32, tag="kcos")
                nc.vector.memset(ktop[64:128].bitcast(F32), 0.0)
                nc.vector.memset(kbot[0:64].bitcast(F32), 0.0)
                nc.vector.memset(vaug[:, :, D:D + 1], 1.0)
                # stream all kv-proj matmuls first (DMA-paced); rope DVE
                # work trails per block, swap-matmuls/transposes after
                for l in range(NL):
                    xkv = xin.tile([128, FCH, LQ], BF16, tag="x")
                    if l == 0:
                        for f in range(FCH):
                            nc.sync.dma_start(xkv[:, f, :],
                                              t["xkv_t"].ap()[l, f])
                    else:
                        nc.sync.dma_start(
                            xkv, t["xkv_t"].ap()[l].rearrange("f p l -> p f l"))
                    pkv = pskv.tile([128, LQ], F32, tag="acc")
                    for f in range(FCH):
                        nc.tensor.matmul(pkv, wkv_sb[:, f, :], xkv[:, f, :],
                                         start=(f == 0), stop=(f == FCH - 1))
                    lb = ts(l, LQ)
                    nc.vector.tensor_scalar_add(kvraw[:, lb], pkv,
                                                bkv_sb[:, 0:1])
                    nc.vector.tensor_mul(tmk[:, lb], kvraw[0:64, lb],
                                         sk[:, lb])
                    nc.vector.tensor_mul(kc[:, lb], kvraw[0:64, lb],
                                         ck[:, lb])
                for l in range(NL):
                    lb = ts(l, LQ)
                    pswk = pskv.tile([128, LQ], F32, tag="acc")
                    nc.tensor.matmul(pswk[0:64], swp[0:64, 0:64],
                                     tmk[:, lb], start=True, stop=True)
                    nc.vector.tensor_add(ktop[0:64, lb],
                                         kc[:, lb], pswk[0:64])
                    nc.sync.dma_start(kbot[64:128, lb], ktop[0:64, lb])
                for c in range(KCH):
                    tp = pskv.tile([128, 512], F32, tag="acc")
                    nc.tensor.transpose(tp[:, 0:64],
                                        kvraw[64:128, ts(c, 128)],
                                        idt[64:128, 64:128])
                    nc.vector.tensor_copy(vaug[:, c, 0:D], tp[:, 0:64])

            # ============ phase B/C: q proj (interleaved) + attention ====
            with (
                tc.tile_pool(name="xq", bufs=1) as xqp,
                tc.tile_pool(name="wst2", bufs=2) as wst2,
                tc.tile_pool(name="ropetmp", bufs=2) as rtp,
                tc.tile_pool(name="ptiles", bufs=3) as ptp,
                tc.tile_pool(name="norm", bufs=2) as nrm,
                tc.tile_pool(name="psq", bufs=2, space="PSUM") as psqp,
                tc.tile_pool(name="psst", bufs=2, space="PSUM") as psst,
                tc.tile_pool(name="oacc", bufs=1, space="PSUM") as oacc,
            ):
                xq = xqp.tile([128, FCH, LQ], BF16)
                nc.scalar.dma_start(
                    xq, t["xq_t"].ap().rearrange("f p l -> p f l"))

                def qproj(j):
                    wq_j = wst2.tile([128, FCH, 128], BF16, tag="w")
                    nc.gpsimd.dma_start(wq_j, t["wq"].ap()[j])
                    psq = psqp.tile([128, LQ], F32, tag="acc")
                    for f in range(FCH):
                        nc.tensor.matmul(psq, wq_j[:, f, :], xq[:, f, :],
                                         start=(f == 0), stop=(f == FCH - 1))
                    # tmq = (psq + bq)*sin ; qc = (psq + bq)*cos  (fused stt;
                    # must be DVE - gpsimd cannot read PSUM)
                    tmq = rtp.tile([128, LQ], F32R, tag="qsin")
                    nc.vector.scalar_tensor_tensor(
                        tmq, psq, bq_sb[:, j:j + 1], sq,
                        mybir.AluOpType.add, mybir.AluOpType.mult)
                    qc = rtp.tile([128, LQ], F32, tag="qcos")
                    nc.vector.scalar_tensor_tensor(
                        qc, psq, bq_sb[:, j:j + 1], cq,
                        mybir.AluOpType.add, mybir.AluOpType.mult)
                    psw = psqp.tile([128, LQ], F32, tag="acc")
                    nc.tensor.matmul(psw, swp, tmq, start=True, stop=True)
                    nc.vector.tensor_add(qrt[j], qc, psw)

                def attn_o(oab, c, pt):
                    nc.tensor.matmul(oab[0:D + 1, 0, :], vaug[:, c, :],
                                     pt[:, 0, :], start=(c == 0),
                                     stop=(c == KCH - 1))
                    nc.tensor.matmul(oab[0:D + 1, 1, :], vaug[:, c, :],
                                     pt[:, 1, :], start=(c == 0),
                                     stop=(c == KCH - 1))

                def norm_finish(j, oraw, den, last):
                    """recip + broadcast + normalize for pair j. Deferred
                    into pair j+1's mask stream so the DVE recip never
                    stalls on the den DMA; broadcast and muls run on the
                    gpsimd queue (SBUF-only). The last pair uses DVE muls
                    for minimum latency into phase D."""
                    rec = nrm.tile([1, 2, LQ], F32, tag="rec")
                    nc.vector.reciprocal_approx_fast(rec, den)
                    rbA = nrm.tile([64, LQ], F32, tag="rbA")
                    rbB = nrm.tile([64, LQ], F32, tag="rbB")
                    nc.gpsimd.partition_broadcast(rbA, rec[0:1, 0, :])
                    nc.gpsimd.partition_broadcast(rbB, rec[0:1, 1, :])
                    eng = nc.vector if last else nc.gpsimd
                    eng.tensor_mul(obt[j][0:64, :], oraw[0:64, 0, :], rbA)
                    osb = nrm.tile([64, LQ], BF16, tag="osb")
                    eng.tensor_mul(osb, oraw[0:64, 1, :], rbB)
                    nc.gpsimd.dma_start(obt[j][64:128, :], osb)
                    if debug and j == 0:
                        nc.sync.dma_start(dbg["d_oraw0"].ap(), oraw)
                        nc.sync.dma_start(dbg["d_rb0"].ap()[:, 0, :], rbA)
                        nc.sync.dma_start(dbg["d_rb0"].ap()[:, 1, :], rbB)
                        nc.sync.dma_start(dbg["d_rec0"].ap(), rec)

                qproj(0)
                # mask DMA behind wq0 on the gpsimd queue: needed only when
                # the first exp output is masked (~20us in)
                nc.gpsimd.dma_start(
                    mt, t["mask_t"].ap().rearrange("c p l -> p c l"))
                pending = None
                for j in range(PAIRS):
                    if j == 3:
                        # Wo preload on the now-idle sync queue; streams
                        # during attention, needed only in phase D.
                        nc.sync.dma_start(wos, t["wo"].ap().rearrange(
                            "fb p j m -> p fb j m"))
                    oab = oacc.tile([128, 2, LQ], F32, tag="oab")
                    prev = None
                    for c in range(KCH):
                        st = psst.tile([128, 2, LQ], F32, tag="st")
                        nc.tensor.matmul(st[:, 0, :], ktop[:, ts(c, 128)],
                                         qrt[j], start=True, stop=True)
                        nc.tensor.matmul(st[:, 1, :], kbot[:, ts(c, 128)],
                                         qrt[j], start=True, stop=True)
                        pt = ptp.tile([128, 2, LQ], FP16, tag="p")
                        nc.scalar.activation(pt, st,
                                             mybir.ActivationFunctionType.Exp)
                        nc.vector.tensor_mul(
                            pt, pt,
                            mt[:, c:c + 1, :].broadcast_to([128, 2, LQ]))
                        if debug and j == 0 and c == 0:
                            nc.sync.dma_start(dbg["d_pt0"].ap(), pt)
                        if c == 2 and pending is not None:
                            norm_finish(*pending, last=False)
                            pending = None
                        if c == 4 and j + 1 < PAIRS:
                            qproj(j + 1)
                        # O matmuls run one chunk behind so they never make
                        # the PE wait on exp+mask of the current chunk.
                        if prev is not None:
                            attn_o(oab, c - 1, prev)
                        prev = pt
                    attn_o(oab, KCH - 1, prev)
                    # stage O+denom to SBUF (frees the PSUM accumulator),
                    # ship the denom row to partition 0, defer the rest.
                    oraw = nrm.tile([D + 1, 2, LQ], F32, tag="oraw")
                    nc.vector.tensor_copy(oraw, oab[0:D + 1, :, :])
                    den = nrm.tile([1, 2, LQ], F32, tag="den")
                    nc.gpsimd.dma_start(den, oraw[D:D + 1, :, :])
                    pending = (j, oraw, den)
                norm_finish(*pending, last=True)

            if debug:
                for j in range(PAIRS):
                    nc.sync.dma_start(dbg["d_qrot"].ap()[:, j, :],
                                      qrt[j].bitcast(F32))
                    nc.sync.dma_start(dbg["d_obig"].ap()[:, j, :], obt[j])
                nc.sync.dma_start(dbg["d_ktop"].ap(), ktop.bitcast(F32))
                nc.sync.dma_start(dbg["d_kbot"].ap(), kbot.bitcast(F32))
                nc.sync.dma_start(dbg["d_vaug16"].ap(), vaug)

            # ================= phase D: output projection =================
            # all j<=6 partial accumulations first (they only need the first
            # 7 pairs' obt, so they overlap the last pair's normalization);
            # then the j=7 finish + bias + store per f-block.
            with (
                tc.tile_pool(name="yout", bufs=2) as yout,
                tc.tile_pool(name="psy", bufs=1, space="PSUM") as psyp,
            ):
                psys = [psyp.tile([128, LQ], F32, tag=f"y{fb}",
                                  name=f"psy{fb}")
                        for fb in range(FCH)]
                for fb in range(FCH):
                    for j in range(FCH - 1):
                        nc.tensor.matmul(psys[fb], wos[:, fb, j, :], obt[j],
                                         start=(j == 0), stop=False)
                for fb in range(FCH):
                    nc.tensor.matmul(psys[fb], wos[:, fb, FCH - 1, :],
                                     obt[FCH - 1], start=False, stop=True)
                    ysb = yout.tile([128, LQ], F32, tag="y")
                    nc.vector.tensor_scalar_add(ysb, psys[fb],
                                                bo_sb[:, fb:fb + 1])
                    nc.sync.dma_start(yT.ap()[ts(fb, 128), :], ysb)

    nc.compile()
    return nc


def _tables():
    """RoPE tables in halves-permuted basis: rows i (even-half) hold +sin,
    rows 32+i (odd-half) hold -sin (for the tmp-then-swap formulation)."""
    inv_freq = 1.0 / (10000.0 ** (np.arange(0, D, 2, dtype=np.float64) / D))
    ang = np.outer(inv_freq, np.arange(L, dtype=np.float64))  # [32, L]
    cos = np.cos(ang).astype(np.float32)
    sin = np.sin(ang).astype(np.float32)
    cos64 = np.concatenate([cos, cos], axis=0)                # [64, L]
    sin_sgn = np.concatenate([sin, -sin], axis=0)             # [64, L]
    return cos64, sin_sgn


def _prep_weights(Wq, bq, Wk, bk, Wv, bv, Wo, bo):
    perm = np.concatenate([np.arange(0, D, 2), np.arange(1, D, 2)])
    WqP = np.asarray(Wq, dtype=np.float32)[:, :, perm].reshape(F, H * D)
    bqP = np.asarray(bq, dtype=np.float32)[:, perm].reshape(H * D)
    WkP = np.asarray(Wk, dtype=np.float32)[:, perm]
    bkP = np.asarray(bk, dtype=np.float32)[perm]
    Wkv = np.concatenate([WkP, np.asarray(Wv, dtype=np.float32)], axis=1)
    bkv = np.concatenate([bkP, np.asarray(bv, dtype=np.float32)])
    WoR = np.asarray(Wo, dtype=np.float32).reshape(H * D, F)
    bo_ = np.asarray(bo, dtype=np.float32)

    wq_pret = np.ascontiguousarray(
        WqP.reshape(FCH, 128, FCH, 128).transpose(2, 1, 0, 3)).astype(
        ml_dtypes.bfloat16)
    wkv_pret = np.ascontiguousarray(
        Wkv.reshape(FCH, 128, 128).transpose(1, 0, 2)).astype(
        ml_dtypes.bfloat16)
    wo_pret = np.ascontiguousarray(
        WoR.reshape(FCH, 128, FCH, 128).transpose(2, 1, 0, 3)).astype(
        ml_dtypes.bfloat16)
    bqbo = np.ascontiguousarray(np.concatenate(
        [bqP.reshape(FCH, 128).T, bo_.reshape(FCH, 128).T], axis=1))
    return wq_pret, wkv_pret, wo_pret, bqbo, bkv


def kernel(inputs_q, inputs_kv, mask, Wq, bq, Wk, bk, Wv, bv, Wo, bo):
    if "nc" not in _CACHED:
        _CACHED["nc"] = build_nc()
    nc = _CACHED["nc"]

    wq_pret, wkv_pret, wo_pret, bqbo, bkv = _prep_weights(
        Wq, bq, Wk, bk, Wv, bv, Wo, bo)

    cos64, sin_sgn = _tables()
    scale = 1.0 / np.sqrt(np.float32(D))
    cksk = np.ascontiguousarray(
        np.concatenate([cos64, sin_sgn], axis=1))      # [64, 2*L] (L=LK)
    cosq_full = np.tile(cos64 * scale, (2, 1))         # [128, L]
    sinq_full = np.tile(sin_sgn * scale, (2, 1))

    xq = np.asarray(inputs_q, dtype=np.float32)
    xkv = np.asarray(inputs_kv, dtype=np.float32)
    mk = np.asarray(mask)

    in_maps = []
    for core in range(NCORES):
        b = core // 4
        qs = (core % 4) * LQ
        xq_t = np.ascontiguousarray(
            xq[b, qs:qs + LQ, :].T.reshape(FCH, 128, LQ)).astype(
            ml_dtypes.bfloat16)
        xkv_t = np.ascontiguousarray(
            xkv[b].T.reshape(FCH, 128, NL, LQ).transpose(2, 0, 1, 3)).astype(
            ml_dtypes.bfloat16)
        mask_t = np.ascontiguousarray(
            mk[b, 0, qs:qs + LQ, :].T.reshape(KCH, 128, LQ)
            .astype(np.float16))
        in_maps.append({
            "xq_t": xq_t,
            "xkv_t": xkv_t,
            "mask_t": mask_t,
            "wq": wq_pret,
            "wkv": wkv_pret,
            "wo": wo_pret,
            "bqbo": bqbo,
            "bkv": bkv,
            "cosq": np.ascontiguousarray(cosq_full[:, qs:qs + LQ]),
            "sinq": np.ascontiguousarray(sinq_full[:, qs:qs + LQ]),
            "cksk": cksk,
        })

    res = bass_utils.run_bass_kernel_spmd(nc, in_maps,
                                          core_ids=list(range(NCORES)))
    _CACHED["last_results"] = res
    _CACHED["last_maps"] = in_maps

    out = np.empty((B, L, F), dtype=np.float32)
    for core in range(NCORES):
        b = core // 4
        qs = (core % 4) * LQ
        out[b, qs:qs + LQ, :] = res.results[core]["yT"].T
    return out


# revision 42
# speedup vs baseline: 1.1439x; 1.0286x over previous
"""MQA attention (B=2, Lq=Lkv=2048, F=1024, H=16, D=64) on 8 TRN2 cores.

Sharding: core = (batch, query-block-of-512). Each core computes its full
output rows (all 16 heads + output projection) -> no collectives; host
concatenates per-core yT slabs.

Per-core dataflow (matmuls in f32r = fp32 rounded to 11-bit mantissa, full
PE rate at >=256 moving cols; only input-rounding error ~1e-4):
  kvT[kd|vd,lk]= Wkv.T @ xkvT   (kv projection FIRST so attention can start)
  RoPE in a halves-permuted head-dim basis (host permutes Wq/Wk columns):
  rot(x) = x*cos + Swap @ (x*sin_signed), Swap a 128x128 PE permutation.
  Attention per head-pair j (q proj for pair j+1 interleaved):
    S^T[lk,lq] = k-chunk.T @ qT  (two 1-bank matmuls into a 2-bank PSUM st)
    P = exp(S^T) [ACT, 2-bank supertile] * maskT [one DVE mult, partition-
    broadcast AP over the two banks]
    O_aug^T += V_aug-chunk.T @ P  (ones column -> row 64 = softmax denom)
    normalize: denom rows -> SBUF (DMA), reciprocal_approx_fast (DVE),
    gpsimd partition_broadcast, two DVE mults (no PE involvement).
  yT[f,lq] = Wo-chunks.T @ obig (+bo), Wo preloaded during attention.
"""

import ml_dtypes
import numpy as np

import concourse.bass as bass
import concourse.tile as tile
from concourse import bacc, mybir
from concourse import bass_utils
from concourse.bass import ts
from concourse.masks import make_identity

F32 = mybir.dt.float32
F32R = mybir.dt.float32r
FP16 = mybir.dt.float16
BF16 = mybir.dt.bfloat16

B, L, F, H, D = 2, 2048, 1024, 16, 64
LQ = 512            # query rows per core
LK = 2048           # kv rows (full)
NCORES = 8
PAIRS = H // 2      # head pairs (one qT partition block each)
FCH = F // 128      # f contraction chunks
KCH = LK // 128     # lk chunks
NL = LK // LQ       # kv column blocks

_CACHED = {}


def round_f32r(x: np.ndarray) -> np.ndarray:
    """Round-to-nearest-even fp32 -> fp32r (11-bit stored mantissa)."""
    u = np.ascontiguousarray(x, dtype=np.float32).view(np.uint32)
    lsb = (u >> np.uint32(12)) & np.uint32(1)
    u2 = (u + np.uint32(0x7FF) + lsb) & np.uint32(0xFFFFF000)
    return u2.view(np.float32)


def build_nc(debug=False):
    nc = bacc.Bacc("TRN2", target_bir_lowering=False, debug=False,
                   num_devices=NCORES)
    dt_in = [
        ("xq_t", [FCH, 128, LQ], BF16),        # [f, p, lq]
        ("xkv_t", [NL, FCH, 128, LQ], BF16),   # [l, f, p, lq]
        ("mask_t", [KCH, 128, LQ], FP16),      # [c, p, lq]
        ("wq", [FCH, 128, FCH, 128], BF16),    # [j, p, f, m]
        ("wkv", [128, FCH, 128], BF16),        # [p, f, m]
        ("wo", [FCH, 128, FCH, 128], BF16),    # [fb, p, j, m]
        ("bqbo", [128, 2 * FCH], F32),         # cols 0:8 bq-blocks, 8:16 bo
        ("bkv", [2 * D], F32),
        ("cosq", [128, LQ], F32),
        ("sinq", [128, LQ], F32),
        ("cksk", [D, 2 * LK], F32),            # [p, (cos|sin)*lk]
    ]
    t = {name: nc.dram_tensor(name, shape, dt, kind="ExternalInput")
         for name, shape, dt in dt_in}
    yT = nc.dram_tensor("yT", [F, LQ], F32, kind="ExternalOutput")
    dbg = {}
    if debug:
        for name, shape in [("d_qrot", [128, PAIRS, LQ]),
                            ("d_ktop", [128, LK]), ("d_kbot", [128, LK]),
                            ("d_vaug", [128, KCH, D + 1]),
                            ("d_oraw0", [D + 1, 2, LQ])]:
            dbg[name] = nc.dram_tensor(name, shape, F32, kind="ExternalOutput")
        dbg["d_pt0"] = nc.dram_tensor("d_pt0", [128, 2, LQ], FP16,
                                      kind="ExternalOutput")
        dbg["d_vaug16"] = nc.dram_tensor("d_vaug16", [128, KCH, D + 1], FP16,
                                         kind="ExternalOutput")
        dbg["d_rb0"] = nc.dram_tensor("d_rb0", [64, 2, LQ], F32,
                                      kind="ExternalOutput")
        dbg["d_rec0"] = nc.dram_tensor("d_rec0", [1, 2, LQ], F32,
                                       kind="ExternalOutput")
        dbg["d_obig"] = nc.dram_tensor("d_obig", [128, PAIRS, LQ], BF16,
                                       kind="ExternalOutput")

    with tile.TileContext(nc) as tc:
        with (
            tc.tile_pool(name="persist", bufs=1) as persist,
            tc.tile_pool(name="small", bufs=4) as small,
        ):
            # ---------- persistent SBUF ----------
            # per-pair tiles (vs one big tile) so the Tile dep-tracker sees
            # no false write-after-read hazards between pairs
            qrt = [persist.tile([128, LQ], F32R, tag=f"qr{j}",
                                name=f"qrt{j}")
                   for j in range(PAIRS)]                 # rotated qT
            obt = [persist.tile([128, LQ], BF16, tag=f"ob{j}",
                                name=f"obt{j}")
                   for j in range(PAIRS)]                 # normalized O^T
            ktop = persist.tile([128, LK], F32R)          # k in rows 0:64
            kbot = persist.tile([128, LK], F32R)          # k in rows 64:128
            vaug = persist.tile([128, KCH, D + 1], FP16)  # V chunks + ones
            mt = persist.tile([128, KCH, LQ], FP16)       # maskT resident
            cq = persist.tile([128, LQ], F32)
            sq = persist.tile([128, LQ], F32)
            cksk = persist.tile([D, 2, LK], F32)
            wos = persist.tile([128, FCH, FCH, 128], BF16)  # Wo resident
            wkv_sb = persist.tile([128, FCH, 128], BF16)

            # scalar queue priority order: wkv (first kv matmul needs it),
            # then the small tables, then xq (issued in phase B).
            nc.scalar.dma_start(wkv_sb, t["wkv"].ap())
            nc.scalar.dma_start(cksk,
                                t["cksk"].ap().rearrange("p (a l) -> p a l",
                                                         a=2))
            nc.scalar.dma_start(cq, t["cosq"].ap())
            nc.scalar.dma_start(sq, t["sinq"].ap())
            bqbo = small.tile([128, 2 * FCH], F32, tag="bias")
            nc.scalar.dma_start(bqbo, t["bqbo"].ap())
            bq_sb = bqbo[:, 0:FCH]
            bo_sb = bqbo[:, FCH:2 * FCH]
            bkv_sb = small.tile([128, 1], F32, tag="bias2")
            nc.scalar.dma_start(bkv_sb, t["bkv"].ap().unsqueeze(1))

            idt = small.tile([128, 128], F32, tag="ident")
            make_identity(nc, idt)
            # halves-swap permutation matrix: M[p, p-xor-32-within-head] = 1
            swpf = small.tile([128, 128], F32, tag="swpf")
            nc.gpsimd.memset(swpf, 0.0)
            for o1, o2 in ((0, 32), (32, 0), (64, 96), (96, 64)):
                nc.gpsimd.affine_select(
                    out=swpf[o1:o1 + 32, o2:o2 + 32],
                    in_=swpf[o1:o1 + 32, o2:o2 + 32],
                    compare_op=mybir.AluOpType.not_equal, fill=1.0,
                    base=0, pattern=[[-1, 32]], channel_multiplier=1)
            swp = small.tile([128, 128], F32R, tag="swp")
            nc.vector.tensor_copy(swp, swpf)

            # ================= phase A: kv projection + rope-k ===========
            with (
                tc.tile_pool(name="xin", bufs=4) as xin,
                tc.tile_pool(name="kvraw", bufs=1) as kvp,
                tc.tile_pool(name="ktmp", bufs=1) as ktp,
                tc.tile_pool(name="pskv", bufs=2, space="PSUM") as pskv,
            ):
                kvraw = kvp.tile([128, LK], F32)
                ck = cksk[:, 0, :]
                sk = cksk[:, 1, :]
                tmk = ktp.tile([D, LK], F32R, tag="ksin")
                kc = ktp.tile([D, LK], F32, tag="kcos")
                nc.vector.memset(ktop[64:128].bitcast(F32), 0.0)
                nc.vector.memset(kbot[0:64].bitcast(F32), 0.0)
                nc.vector.memset(vaug[:, :, D:D + 1], 1.0)
                # stream all kv-proj matmuls first (DMA-paced); rope DVE
                # work trails per block, swap-matmuls/transposes after
                for l in range(NL):
                    xkv = xin.tile([128, FCH, LQ], BF16, tag="x")
                    if l == 0:
                        for f in range(FCH):
                            nc.sync.dma_start(xkv[:, f, :],
                                              t["xkv_t"].ap()[l, f])
                    else:
                        nc.sync.dma_start(
                            xkv, t["xkv_t"].ap()[l].rearrange("f p l -> p f l"))
                    pkv = pskv.tile([128, LQ], F32, tag="acc")
                    for f in range(FCH):
                        nc.tensor.matmul(pkv, wkv_sb[:, f, :], xkv[:, f, :],
                                         start=(f == 0), stop=(f == FCH - 1))
                    lb = ts(l, LQ)
                    nc.vector.tensor_scalar_add(kvraw[:, lb], pkv,
                                                bkv_sb[:, 0:1])
                    nc.vector.tensor_mul(tmk[:, lb], kvraw[0:64, lb],
                                         sk[:, lb])
                    nc.vector.tensor_mul(kc[:, lb], kvraw[0:64, lb],
                                         ck[:, lb])
                nc.sync.dma_start(
                    mt, t["mask_t"].ap().rearrange("c p l -> p c l"))
                for l in range(NL):
                    lb = ts(l, LQ)
                    pswk = pskv.tile([128, LQ], F32, tag="acc")
                    nc.tensor.matmul(pswk[0:64], swp[0:64, 0:64],
                                     tmk[:, lb], start=True, stop=True)
                    nc.vector.tensor_add(ktop[0:64, lb],
                                         kc[:, lb], pswk[0:64])
                    nc.sync.dma_start(kbot[64:128, lb], ktop[0:64, lb])
                for c in range(KCH):
                    tp = pskv.tile([128, 512], F32, tag="acc")
                    nc.tensor.transpose(tp[:, 0:64],
                                        kvraw[64:128, ts(c, 128)],
                                        idt[64:128, 64:128])
                    nc.vector.tensor_copy(vaug[:, c, 0:D], tp[:, 0:64])

            # ============ phase B/C: q proj (interleaved) + attention ====
            with (
                tc.tile_pool(name="xq", bufs=1) as xqp,
                tc.tile_pool(name="wst2", bufs=2) as wst2,
                tc.tile_pool(name="ropetmp", bufs=2) as rtp,
                tc.tile_pool(name="ptiles", bufs=3) as ptp,
                tc.tile_pool(name="norm", bufs=2) as nrm,
                tc.tile_pool(name="psq", bufs=2, space="PSUM") as psqp,
                tc.tile_pool(name="psst", bufs=2, space="PSUM") as psst,
                tc.tile_pool(name="oacc", bufs=1, space="PSUM") as oacc,
            ):
                xq = xqp.tile([128, FCH, LQ], BF16)
                nc.scalar.dma_start(
                    xq, t["xq_t"].ap().rearrange("f p l -> p f l"))

                def qproj(j):
                    wq_j = wst2.tile([128, FCH, 128], BF16, tag="w")
                    nc.gpsimd.dma_start(wq_j, t["wq"].ap()[j])
                    psq = psqp.tile([128, LQ], F32, tag="acc")
                    for f in range(FCH):
                        nc.tensor.matmul(psq, wq_j[:, f, :], xq[:, f, :],
                                         start=(f == 0), stop=(f == FCH - 1))
                    # tmq = (psq + bq)*sin ; qc = (psq + bq)*cos  (fused stt;
                    # must be DVE - gpsimd cannot read PSUM)
                    tmq = rtp.tile([128, LQ], F32R, tag="qsin")
                    nc.vector.scalar_tensor_tensor(
                        tmq, psq, bq_sb[:, j:j + 1], sq,
                        mybir.AluOpType.add, mybir.AluOpType.mult)
                    qc = rtp.tile([128, LQ], F32, tag="qcos")
                    nc.vector.scalar_tensor_tensor(
                        qc, psq, bq_sb[:, j:j + 1], cq,
                        mybir.AluOpType.add, mybir.AluOpType.mult)
                    psw = psqp.tile([128, LQ], F32, tag="acc")
                    nc.tensor.matmul(psw, swp, tmq, start=True, stop=True)
                    nc.vector.tensor_add(qrt[j], qc, psw)

                def attn_o(oab, c, pt):
                    nc.tensor.matmul(oab[0:D + 1, 0, :], vaug[:, c, :],
                                     pt[:, 0, :], start=(c == 0),
                                     stop=(c == KCH - 1))
                    nc.tensor.matmul(oab[0:D + 1, 1, :], vaug[:, c, :],
                                     pt[:, 1, :], start=(c == 0),
                                     stop=(c == KCH - 1))

                def norm_finish(j, oraw, den, last):
                    """recip + broadcast + normalize for pair j. Deferred
                    into pair j+1's mask stream so the DVE recip never
                    stalls on the den DMA; broadcast and muls run on the
                    gpsimd queue (SBUF-only). The last pair uses DVE muls
                    for minimum latency into phase D."""
                    rec = nrm.tile([1, 2, LQ], F32, tag="rec")
                    nc.vector.reciprocal_approx_fast(rec, den)
                    rbA = nrm.tile([64, LQ], F32, tag="rbA")
                    rbB = nrm.tile([64, LQ], F32, tag="rbB")
                    nc.gpsimd.partition_broadcast(rbA, rec[0:1, 0, :])
                    nc.gpsimd.partition_broadcast(rbB, rec[0:1, 1, :])
                    eng = nc.vector if last else nc.gpsimd
                    eng.tensor_mul(obt[j][0:64, :], oraw[0:64, 0, :], rbA)
                    osb = nrm.tile([64, LQ], BF16, tag="osb")
                    eng.tensor_mul(osb, oraw[0:64, 1, :], rbB)
                    nc.gpsimd.dma_start(obt[j][64:128, :], osb)
                    if debug and j == 0:
                        nc.sync.dma_start(dbg["d_oraw0"].ap(), oraw)
                        nc.sync.dma_start(dbg["d_rb0"].ap()[:, 0, :], rbA)
                        nc.sync.dma_start(dbg["d_rb0"].ap()[:, 1, :], rbB)
                        nc.sync.dma_start(dbg["d_rec0"].ap(), rec)

                qproj(0)
                pending_oab = None
                pending_cd = None
                for j in range(PAIRS):
                    if j == 3:
                        # Wo preload on the now-idle sync queue; streams
                        # during attention, needed only in phase D.
                        nc.sync.dma_start(wos, t["wo"].ap().rearrange(
                            "fb p j m -> p fb j m"))
                    oab = oacc.tile([128, 2, LQ], F32, tag="oab")
                    prev = None
                    for c in range(KCH):
                        st = psst.tile([128, 2, LQ], F32, tag="st")
                        nc.tensor.matmul(st[:, 0, :], ktop[:, ts(c, 128)],
                                         qrt[j], start=True, stop=True)
                        nc.tensor.matmul(st[:, 1, :], kbot[:, ts(c, 128)],
                                         qrt[j], start=True, stop=True)
                        pt = ptp.tile([128, 2, LQ], FP16, tag="p")
                        nc.scalar.activation(pt, st,
                                             mybir.ActivationFunctionType.Exp)
                        nc.vector.tensor_mul(
                            pt, pt,
                            mt[:, c:c + 1, :].broadcast_to([128, 2, LQ]))
                        if debug and j == 0 and c == 0:
                            nc.sync.dma_start(dbg["d_pt0"].ap(), pt)
                        if c == 1 and pending_oab is not None:
                            pj, poab = pending_oab
                            pending_oab = None
                            oraw = nrm.tile([D + 1, 2, LQ], F32, tag="oraw",
                                            name="oraw_d")
                            nc.vector.tensor_copy(oraw, poab[0:D + 1, :, :])
                            den = nrm.tile([1, 2, LQ], F32, tag="den",
                                           name="den_d")
                            nc.gpsimd.dma_start(den, oraw[D:D + 1, :, :])
                            pending_cd = (pj, oraw, den)
                        if c == 4 and pending_cd is not None:
                            norm_finish(*pending_cd, last=False)
                            pending_cd = None
                        if c == 6 and j + 1 < PAIRS:
                            qproj(j + 1)
                        # O matmuls run one chunk behind so they never make
                        # the PE wait on exp+mask of the current chunk.
                        if prev is not None:
                            attn_o(oab, c - 1, prev)
                        prev = pt
                    attn_o(oab, KCH - 1, prev)
                    pending_oab = (j, oab)
                # last pair: immediate staging + normalization (DVE muls)
                pj, poab = pending_oab
                oraw = nrm.tile([D + 1, 2, LQ], F32, tag="oraw",
                                name="oraw_l")
                nc.vector.tensor_copy(oraw, poab[0:D + 1, :, :])
                den = nrm.tile([1, 2, LQ], F32, tag="den", name="den_l")
                nc.gpsimd.dma_start(den, oraw[D:D + 1, :, :])
                norm_finish(pj, oraw, den, last=True)

            if debug:
                for j in range(PAIRS):
                    nc.sync.dma_start(dbg["d_qrot"].ap()[:, j, :],
                                      qrt[j].bitcast(F32))
                    nc.sync.dma_start(dbg["d_obig"].ap()[:, j, :], obt[j])
                nc.sync.dma_start(dbg["d_ktop"].ap(), ktop.bitcast(F32))
                nc.sync.dma_start(dbg["d_kbot"].ap(), kbot.bitcast(F32))
                nc.sync.dma_start(dbg["d_vaug16"].ap(), vaug)

            # ================= phase D: output projection =================
            # all j<=6 partial accumulations first (they only need the first
            # 7 pairs' obt, so they overlap the last pair's normalization);
            # then the j=7 finish + bias + store per f-block.
            with (
                tc.tile_pool(name="yout", bufs=2) as yout,
                tc.tile_pool(name="psy", bufs=1, space="PSUM") as psyp,
            ):
                psys = [psyp.tile([128, LQ], F32, tag=f"y{fb}",
                                  name=f"psy{fb}")
                        for fb in range(FCH)]
                for fb in range(FCH):
                    for j in range(FCH - 1):
                        nc.tensor.matmul(psys[fb], wos[:, fb, j, :], obt[j],
                                         start=(j == 0), stop=False)
                for fb in range(FCH):
                    nc.tensor.matmul(psys[fb], wos[:, fb, FCH - 1, :],
                                     obt[FCH - 1], start=False, stop=True)
                    ysb = yout.tile([128, LQ], F32, tag="y")
                    nc.vector.tensor_scalar_add(ysb, psys[fb],
                                                bo_sb[:, fb:fb + 1])
                    nc.sync.dma_start(yT.ap()[ts(fb, 128), :], ysb)

    nc.compile()
    return nc


def _tables():
    """RoPE tables in halves-permuted basis: rows i (even-half) hold +sin,
    rows 32+i (odd-half) hold -sin (for the tmp-then-swap formulation)."""
    inv_freq = 1.0 / (10000.0 ** (np.arange(0, D, 2, dtype=np.float64) / D))
    ang = np.outer(inv_freq, np.arange(L, dtype=np.float64))  # [32, L]
    cos = np.cos(ang).astype(np.float32)
    sin = np.sin(ang).astype(np.float32)
    cos64 = np.concatenate([cos, cos], axis=0)                # [64, L]
    sin_sgn = np.concatenate([sin, -sin], axis=0)             # [64, L]
    return cos64, sin_sgn


def _prep_weights(Wq, bq, Wk, bk, Wv, bv, Wo, bo):
    perm = np.concatenate([np.arange(0, D, 2), np.arange(1, D, 2)])
    WqP = np.asarray(Wq, dtype=np.float32)[:, :, perm].reshape(F, H * D)
    bqP = np.asarray(bq, dtype=np.float32)[:, perm].reshape(H * D)
    WkP = np.asarray(Wk, dtype=np.float32)[:, perm]
    bkP = np.asarray(bk, dtype=np.float32)[perm]
    Wkv = np.concatenate([WkP, np.asarray(Wv, dtype=np.float32)], axis=1)
    bkv = np.concatenate([bkP, np.asarray(bv, dtype=np.float32)])
    WoR = np.asarray(Wo, dtype=np.float32).reshape(H * D, F)
    bo_ = np.asarray(bo, dtype=np.float32)

    wq_pret = np.ascontiguousarray(
        WqP.reshape(FCH, 128, FCH, 128).transpose(2, 1, 0, 3)).astype(
        ml_dtypes.bfloat16)
    wkv_pret = np.ascontiguousarray(
        Wkv.reshape(FCH, 128, 128).transpose(1, 0, 2)).astype(
        ml_dtypes.bfloat16)
    wo_pret = np.ascontiguousarray(
        WoR.reshape(FCH, 128, FCH, 128).transpose(2, 1, 0, 3)).astype(
        ml_dtypes.bfloat16)
    bqbo = np.ascontiguousarray(np.concatenate(
        [bqP.reshape(FCH, 128).T, bo_.reshape(FCH, 128).T], axis=1))
    return wq_pret, wkv_pret, wo_pret, bqbo, bkv


def kernel(inputs_q, inputs_kv, mask, Wq, bq, Wk, bk, Wv, bv, Wo, bo):
    if "nc" not in _CACHED:
        _CACHED["nc"] = build_nc()
    nc = _CACHED["nc"]

    wq_pret, wkv_pret, wo_pret, bqbo, bkv = _prep_weights(
        Wq, bq, Wk, bk, Wv, bv, Wo, bo)

    cos64, sin_sgn = _tables()
    scale = 1.0 / np.sqrt(np.float32(D))
    cksk = np.ascontiguousarray(
        np.concatenate([cos64, sin_sgn], axis=1))      # [64, 2*L] (L=LK)
    cosq_full = np.tile(cos64 * scale, (2, 1))         # [128, L]
    sinq_full = np.tile(sin_sgn * scale, (2, 1))

    xq = np.asarray(inputs_q, dtype=np.float32)
    xkv = np.asarray(inputs_kv, dtype=np.float32)
    mk = np.asarray(mask)

    in_maps = []
    for core in range(NCORES):
        b = core // 4
        qs = (core % 4) * LQ
        xq_t = np.ascontiguousarray(
            xq[b, qs:qs + LQ, :].T.reshape(FCH, 128, LQ)).astype(
            ml_dtypes.bfloat16)
        xkv_t = np.ascontiguousarray(
            xkv[b].T.reshape(FCH, 128, NL, LQ).transpose(2, 0, 1, 3)).astype(
            ml_dtypes.bfloat16)
        mask_t = np.ascontiguousarray(
            mk[b, 0, qs:qs + LQ, :].T.reshape(KCH, 128, LQ)
            .astype(np.float16))
        in_maps.append({
            "xq_t": xq_t,
            "xkv_t": xkv_t,
            "mask_t": mask_t,
            "wq": wq_pret,
            "wkv": wkv_pret,
            "wo": wo_pret,
            "bqbo": bqbo,
            "bkv": bkv,
            "cosq": np.ascontiguousarray(cosq_full[:, qs:qs + LQ]),
            "sinq": np.ascontiguousarray(sinq_full[:, qs:qs + LQ]),
            "cksk": cksk,
        })

    res = bass_utils.run_bass_kernel_spmd(nc, in_maps,
                                          core_ids=list(range(NCORES)))
    _CACHED["last_results"] = res
    _CACHED["last_maps"] = in_maps

    out = np.empty((B, L, F), dtype=np.float32)
    for core in range(NCORES):
        b = core // 4
        qs = (core % 4) * LQ
        out[b, qs:qs + LQ, :] = res.results[core]["yT"].T
    return out


# revision 43
# speedup vs baseline: 1.1796x; 1.0312x over previous
"""MQA attention (B=2, Lq=Lkv=2048, F=1024, H=16, D=64) on 8 TRN2 cores.

Sharding: core = (batch, query-block-of-512). Each core computes its full
output rows (all 16 heads + output projection) -> no collectives; host
concatenates per-core yT slabs.

Per-core dataflow (matmuls in f32r = fp32 rounded to 11-bit mantissa, full
PE rate at >=256 moving cols; only input-rounding error ~1e-4):
  kvT[kd|vd,lk]= Wkv.T @ xkvT   (kv projection FIRST so attention can start)
  RoPE in a halves-permuted head-dim basis (host permutes Wq/Wk columns):
  rot(x) = x*cos + Swap @ (x*sin_signed), Swap a 128x128 PE permutation.
  Attention per head-pair j (q proj for pair j+1 interleaved):
    S^T[lk,lq] = k-chunk.T @ qT  (two 1-bank matmuls into a 2-bank PSUM st)
    P = exp(S^T) [ACT, 2-bank supertile] * maskT [one DVE mult, partition-
    broadcast AP over the two banks]
    O_aug^T += V_aug-chunk.T @ P  (ones column -> row 64 = softmax denom)
    normalize: denom rows -> SBUF (DMA), reciprocal_approx_fast (DVE),
    gpsimd partition_broadcast, two DVE mults (no PE involvement).
  yT[f,lq] = Wo-chunks.T @ obig (+bo), Wo preloaded during attention.
"""

import ml_dtypes
import numpy as np

import concourse.bass as bass
import concourse.tile as tile
from concourse import bacc, mybir
from concourse import bass_utils
from concourse.bass import ts
from concourse.masks import make_identity

F32 = mybir.dt.float32
F32R = mybir.dt.float32r
FP16 = mybir.dt.float16
BF16 = mybir.dt.bfloat16

B, L, F, H, D = 2, 2048, 1024, 16, 64
LQ = 512            # query rows per core
LK = 2048           # kv rows (full)
NCORES = 8
PAIRS = H // 2      # head pairs (one qT partition block each)
FCH = F // 128      # f contraction chunks
KCH = LK // 128     # lk chunks
NL = LK // LQ       # kv column blocks

_CACHED = {}


def round_f32r(x: np.ndarray) -> np.ndarray:
    """Round-to-nearest-even fp32 -> fp32r (11-bit stored mantissa)."""
    u = np.ascontiguousarray(x, dtype=np.float32).view(np.uint32)
    lsb = (u >> np.uint32(12)) & np.uint32(1)
    u2 = (u + np.uint32(0x7FF) + lsb) & np.uint32(0xFFFFF000)
    return u2.view(np.float32)


def build_nc(debug=False):
    nc = bacc.Bacc("TRN2", target_bir_lowering=False, debug=False,
                   num_devices=NCORES)
    dt_in = [
        ("xq_t", [FCH, 128, LQ], BF16),        # [f, p, lq]
        ("xkv_t", [NL, FCH, 128, LQ], BF16),   # [l, f, p, lq]
        ("mask_t", [KCH, 128, LQ], FP16),      # [c, p, lq]
        ("wq", [FCH, 128, FCH, 128], BF16),    # [j, p, f, m]
        ("wkv", [128, FCH, 128], BF16),        # [p, f, m]
        ("wo", [FCH, 128, FCH, 128], BF16),    # [fb, p, j, m]
        ("bqbo", [128, 2 * FCH], F32),         # cols 0:8 bq-blocks, 8:16 bo
        ("bkv", [2 * D], F32),
        ("cosq", [128, LQ], F32),
        ("sinq", [128, LQ], F32),
        ("cksk", [D, 2 * LK], F32),            # [p, (cos|sin)*lk]
    ]
    t = {name: nc.dram_tensor(name, shape, dt, kind="ExternalInput")
         for name, shape, dt in dt_in}
    yT = nc.dram_tensor("yT", [F, LQ], F32, kind="ExternalOutput")
    dbg = {}
    if debug:
        for name, shape in [("d_qrot", [128, PAIRS, LQ]),
                            ("d_ktop", [128, LK]), ("d_kbot", [128, LK]),
                            ("d_vaug", [128, KCH, D + 1]),
                            ("d_oraw0", [D + 1, 2, LQ])]:
            dbg[name] = nc.dram_tensor(name, shape, F32, kind="ExternalOutput")
        dbg["d_pt0"] = nc.dram_tensor("d_pt0", [128, 2, LQ], FP16,
                                      kind="ExternalOutput")
        dbg["d_vaug16"] = nc.dram_tensor("d_vaug16", [128, KCH, D + 1], FP16,
                                         kind="ExternalOutput")
        dbg["d_rb0"] = nc.dram_tensor("d_rb0", [64, 2, LQ], F32,
                                      kind="ExternalOutput")
        dbg["d_rec0"] = nc.dram_tensor("d_rec0", [1, 2, LQ], F32,
                                       kind="ExternalOutput")
        dbg["d_obig"] = nc.dram_tensor("d_obig", [128, PAIRS, LQ], BF16,
                                       kind="ExternalOutput")

    with tile.TileContext(nc) as tc:
        with (
            tc.tile_pool(name="persist", bufs=1) as persist,
            tc.tile_pool(name="small", bufs=4) as small,
        ):
            # ---------- persistent SBUF ----------
            # per-pair tiles (vs one big tile) so the Tile dep-tracker sees
            # no false write-after-read hazards between pairs
            qrt = [persist.tile([128, LQ], F32R, tag=f"qr{j}",
                                name=f"qrt{j}")
                   for j in range(PAIRS)]                 # rotated qT
            obt = [persist.tile([128, LQ], BF16, tag=f"ob{j}",
                                name=f"obt{j}")
                   for j in range(PAIRS)]                 # normalized O^T
            ktop = persist.tile([128, LK], F32R)          # k in rows 0:64
            kbot = persist.tile([128, LK], F32R)          # k in rows 64:128
            vaug = persist.tile([128, KCH, D + 1], FP16)  # V chunks + ones
            mt = persist.tile([128, KCH, LQ], FP16)       # maskT resident
            cq = persist.tile([128, LQ], F32)
            sq = persist.tile([128, LQ], F32)
            cksk = persist.tile([D, 2, LK], F32)
            wos = persist.tile([128, FCH, FCH, 128], BF16)  # Wo resident
            wkv_sb = persist.tile([128, FCH, 128], BF16)

            # scalar queue priority order: wkv (first kv matmul needs it),
            # then the small tables, then xq (issued in phase B).
            nc.scalar.dma_start(wkv_sb, t["wkv"].ap())
            nc.scalar.dma_start(cksk,
                                t["cksk"].ap().rearrange("p (a l) -> p a l",
                                                         a=2))
            nc.scalar.dma_start(cq, t["cosq"].ap())
            nc.scalar.dma_start(sq, t["sinq"].ap())
            bqbo = small.tile([128, 2 * FCH], F32, tag="bias")
            nc.scalar.dma_start(bqbo, t["bqbo"].ap())
            bq_sb = bqbo[:, 0:FCH]
            bo_sb = bqbo[:, FCH:2 * FCH]
            bkv_sb = small.tile([128, 1], F32, tag="bias2")
            nc.scalar.dma_start(bkv_sb, t["bkv"].ap().unsqueeze(1))

            idt = small.tile([128, 128], F32, tag="ident")
            make_identity(nc, idt)
            # halves-swap permutation matrix: M[p, p-xor-32-within-head] = 1
            swpf = small.tile([128, 128], F32, tag="swpf")
            nc.gpsimd.memset(swpf, 0.0)
            for o1, o2 in ((0, 32), (32, 0), (64, 96), (96, 64)):
                nc.gpsimd.affine_select(
                    out=swpf[o1:o1 + 32, o2:o2 + 32],
                    in_=swpf[o1:o1 + 32, o2:o2 + 32],
                    compare_op=mybir.AluOpType.not_equal, fill=1.0,
                    base=0, pattern=[[-1, 32]], channel_multiplier=1)
            swp = small.tile([128, 128], F32R, tag="swp")
            nc.vector.tensor_copy(swp, swpf)

            # ================= phase A: kv projection + rope-k ===========
            with (
                tc.tile_pool(name="xin", bufs=4) as xin,
                tc.tile_pool(name="kvraw", bufs=1) as kvp,
                tc.tile_pool(name="ktmp", bufs=1) as ktp,
                tc.tile_pool(name="pskv", bufs=2, space="PSUM") as pskv,
            ):
                kvraw = kvp.tile([128, LK], F32)
                ck = cksk[:, 0, :]
                sk = cksk[:, 1, :]
                tmk = ktp.tile([D, LK], F32R, tag="ksin")
                kc = ktp.tile([D, LK], F32, tag="kcos")
                nc.vector.memset(ktop[64:128].bitcast(F32), 0.0)
                nc.vector.memset(kbot[0:64].bitcast(F32), 0.0)
                nc.vector.memset(vaug[:, :, D:D + 1], 1.0)
                # stream all kv-proj matmuls first (DMA-paced); rope DVE
                # work trails per block, swap-matmuls/transposes after
                for l in range(NL):
                    xkv = xin.tile([128, FCH, LQ], BF16, tag="x")
                    if l == 0:
                        for f in range(FCH):
                            nc.sync.dma_start(xkv[:, f, :],
                                              t["xkv_t"].ap()[l, f])
                    else:
                        nc.sync.dma_start(
                            xkv, t["xkv_t"].ap()[l].rearrange("f p l -> p f l"))
                    pkv = pskv.tile([128, LQ], F32, tag="acc")
                    for f in range(FCH):
                        nc.tensor.matmul(pkv, wkv_sb[:, f, :], xkv[:, f, :],
                                         start=(f == 0), stop=(f == FCH - 1))
                    lb = ts(l, LQ)
                    nc.vector.tensor_scalar_add(kvraw[:, lb], pkv,
                                                bkv_sb[:, 0:1])
                    nc.vector.tensor_mul(tmk[:, lb], kvraw[0:64, lb],
                                         sk[:, lb])
                    nc.vector.tensor_mul(kc[:, lb], kvraw[0:64, lb],
                                         ck[:, lb])
                nc.sync.dma_start(
                    mt, t["mask_t"].ap().rearrange("c p l -> p c l"))
                for l in range(NL):
                    lb = ts(l, LQ)
                    pswk = pskv.tile([128, LQ], F32, tag="acc")
                    nc.tensor.matmul(pswk[0:64], swp[0:64, 0:64],
                                     tmk[:, lb], start=True, stop=True)
                    nc.vector.tensor_add(ktop[0:64, lb],
                                         kc[:, lb], pswk[0:64])
                    nc.sync.dma_start(kbot[64:128, lb], ktop[0:64, lb])
                for c in range(KCH):
                    tp = pskv.tile([128, 512], F32, tag="acc")
                    nc.tensor.transpose(tp[:, 0:64],
                                        kvraw[64:128, ts(c, 128)],
                                        idt[64:128, 64:128])
                    nc.vector.tensor_copy(vaug[:, c, 0:D], tp[:, 0:64])

            # ============ phase B/C: q proj (interleaved) + attention ====
            with (
                tc.tile_pool(name="xq", bufs=1) as xqp,
                tc.tile_pool(name="wst2", bufs=2) as wst2,
                tc.tile_pool(name="ropetmp", bufs=2) as rtp,
                tc.tile_pool(name="ptiles", bufs=3) as ptp,
                tc.tile_pool(name="norm", bufs=2) as nrm,
                tc.tile_pool(name="psq", bufs=2, space="PSUM") as psqp,
                tc.tile_pool(name="psst", bufs=2, space="PSUM") as psst,
                tc.tile_pool(name="oacc", bufs=1, space="PSUM") as oacc,
            ):
                xq = xqp.tile([128, FCH, LQ], BF16)
                nc.scalar.dma_start(
                    xq, t["xq_t"].ap().rearrange("f p l -> p f l"))

                def qproj(j):
                    wq_j = wst2.tile([128, FCH, 128], BF16, tag="w")
                    nc.gpsimd.dma_start(wq_j, t["wq"].ap()[j])
                    psq = psqp.tile([128, LQ], F32, tag="acc")
                    for f in range(FCH):
                        nc.tensor.matmul(psq, wq_j[:, f, :], xq[:, f, :],
                                         start=(f == 0), stop=(f == FCH - 1))
                    # tmq = (psq + bq)*sin ; qc = (psq + bq)*cos  (fused stt;
                    # must be DVE - gpsimd cannot read PSUM)
                    tmq = rtp.tile([128, LQ], F32R, tag="qsin")
                    nc.vector.scalar_tensor_tensor(
                        tmq, psq, bq_sb[:, j:j + 1], sq,
                        mybir.AluOpType.add, mybir.AluOpType.mult)
                    qc = rtp.tile([128, LQ], F32, tag="qcos")
                    nc.vector.scalar_tensor_tensor(
                        qc, psq, bq_sb[:, j:j + 1], cq,
                        mybir.AluOpType.add, mybir.AluOpType.mult)
                    psw = psqp.tile([128, LQ], F32, tag="acc")
                    nc.tensor.matmul(psw, swp, tmq, start=True, stop=True)
                    nc.vector.tensor_add(qrt[j], qc, psw)

                def attn_o(oab, c, pt):
                    nc.tensor.matmul(oab[0:D + 1, 0, :], vaug[:, c, :],
                                     pt[:, 0, :], start=(c == 0),
                                     stop=(c == KCH - 1))
                    nc.tensor.matmul(oab[0:D + 1, 1, :], vaug[:, c, :],
                                     pt[:, 1, :], start=(c == 0),
                                     stop=(c == KCH - 1))

                def norm_finish(j, oraw, den, last):
                    """recip + broadcast + normalize for pair j. Deferred
                    into pair j+1's mask stream so the DVE recip never
                    stalls on the den DMA; broadcast and muls run on the
                    gpsimd queue (SBUF-only). The last pair uses DVE muls
                    for minimum latency into phase D."""
                    rec = nrm.tile([1, 2, LQ], F32, tag="rec")
                    nc.vector.reciprocal_approx_fast(rec, den)
                    rbA = nrm.tile([64, LQ], F32, tag="rbA")
                    rbB = nrm.tile([64, LQ], F32, tag="rbB")
                    nc.gpsimd.partition_broadcast(rbA, rec[0:1, 0, :])
                    nc.gpsimd.partition_broadcast(rbB, rec[0:1, 1, :])
                    eng = nc.vector if last else nc.gpsimd
                    eng.tensor_mul(obt[j][0:64, :], oraw[0:64, 0, :], rbA)
                    osb = nrm.tile([64, LQ], BF16, tag="osb")
                    eng.tensor_mul(osb, oraw[0:64, 1, :], rbB)
                    nc.gpsimd.dma_start(obt[j][64:128, :], osb)
                    if debug and j == 0:
                        nc.sync.dma_start(dbg["d_oraw0"].ap(), oraw)
                        nc.sync.dma_start(dbg["d_rb0"].ap()[:, 0, :], rbA)
                        nc.sync.dma_start(dbg["d_rb0"].ap()[:, 1, :], rbB)
                        nc.sync.dma_start(dbg["d_rec0"].ap(), rec)

                qproj(0)
                pending_oab = None
                pending_cd = None
                for j in range(PAIRS):
                    if j == 3:
                        # Wo preload on the now-idle sync queue; streams
                        # during attention, needed only in phase D.
                        nc.sync.dma_start(wos, t["wo"].ap().rearrange(
                            "fb p j m -> p fb j m"))
                    oab = oacc.tile([128, 2, LQ], F32, tag="oab")
                    prev = None
                    for c in range(KCH):
                        st = psst.tile([128, 2, LQ], F32, tag="st")
                        nc.tensor.matmul(st[:, 0, :], ktop[:, ts(c, 128)],
                                         qrt[j], start=True, stop=True)
                        nc.tensor.matmul(st[:, 1, :], kbot[:, ts(c, 128)],
                                         qrt[j], start=True, stop=True)
                        pt = ptp.tile([128, 2, LQ], FP16, tag="p")
                        nc.scalar.activation(pt, st,
                                             mybir.ActivationFunctionType.Exp)
                        nc.vector.tensor_mul(
                            pt, pt,
                            mt[:, c:c + 1, :].broadcast_to([128, 2, LQ]))
                        if debug and j == 0 and c == 0:
                            nc.sync.dma_start(dbg["d_pt0"].ap(), pt)
                        if c == 1 and pending_oab is not None:
                            pj, poab = pending_oab
                            pending_oab = None
                            oraw = nrm.tile([D + 1, 2, LQ], F32, tag="oraw",
                                            name="oraw_d")
                            nc.scalar.copy(oraw, poab[0:D + 1, :, :])
                            den = nrm.tile([1, 2, LQ], F32, tag="den",
                                           name="den_d")
                            nc.gpsimd.dma_start(den, oraw[D:D + 1, :, :])
                            pending_cd = (pj, oraw, den)
                        if c == 4 and pending_cd is not None:
                            norm_finish(*pending_cd, last=False)
                            pending_cd = None
                        if c == 6 and j + 1 < PAIRS:
                            qproj(j + 1)
                        # O matmuls run one chunk behind so they never make
                        # the PE wait on exp+mask of the current chunk.
                        if prev is not None:
                            attn_o(oab, c - 1, prev)
                        prev = pt
                    attn_o(oab, KCH - 1, prev)
                    pending_oab = (j, oab)
                # last pair: immediate staging + normalization (DVE muls)
                pj, poab = pending_oab
                oraw = nrm.tile([D + 1, 2, LQ], F32, tag="oraw",
                                name="oraw_l")
                nc.vector.tensor_copy(oraw, poab[0:D + 1, :, :])
                den = nrm.tile([1, 2, LQ], F32, tag="den", name="den_l")
                nc.gpsimd.dma_start(den, oraw[D:D + 1, :, :])
                norm_finish(pj, oraw, den, last=True)

            if debug:
                for j in range(PAIRS):
                    nc.sync.dma_start(dbg["d_qrot"].ap()[:, j, :],
                                      qrt[j].bitcast(F32))
                    nc.sync.dma_start(dbg["d_obig"].ap()[:, j, :], obt[j])
                nc.sync.dma_start(dbg["d_ktop"].ap(), ktop.bitcast(F32))
                nc.sync.dma_start(dbg["d_kbot"].ap(), kbot.bitcast(F32))
                nc.sync.dma_start(dbg["d_vaug16"].ap(), vaug)

            # ================= phase D: output projection =================
            # all j<=6 partial accumulations first (they only need the first
            # 7 pairs' obt, so they overlap the last pair's normalization);
            # then the j=7 finish + bias + store per f-block.
            with (
                tc.tile_pool(name="yout", bufs=2) as yout,
                tc.tile_pool(name="psy", bufs=1, space="PSUM") as psyp,
            ):
                psys = [psyp.tile([128, LQ], F32, tag=f"y{fb}",
                                  name=f"psy{fb}")
                        for fb in range(FCH)]
                for fb in range(FCH):
                    for j in range(FCH - 1):
                        nc.tensor.matmul(psys[fb], wos[:, fb, j, :], obt[j],
                                         start=(j == 0), stop=False)
                for fb in range(FCH):
                    nc.tensor.matmul(psys[fb], wos[:, fb, FCH - 1, :],
                                     obt[FCH - 1], start=False, stop=True)
                    ysb = yout.tile([128, LQ], F32, tag="y")
                    nc.vector.tensor_scalar_add(ysb, psys[fb],
                                                bo_sb[:, fb:fb + 1])
                    nc.sync.dma_start(yT.ap()[ts(fb, 128), :], ysb)

    nc.compile()
    return nc


def _tables():
    """RoPE tables in halves-permuted basis: rows i (even-half) hold +sin,
    rows 32+i (odd-half) hold -sin (for the tmp-then-swap formulation)."""
    inv_freq = 1.0 / (10000.0 ** (np.arange(0, D, 2, dtype=np.float64) / D))
    ang = np.outer(inv_freq, np.arange(L, dtype=np.float64))  # [32, L]
    cos = np.cos(ang).astype(np.float32)
    sin = np.sin(ang).astype(np.float32)
    cos64 = np.concatenate([cos, cos], axis=0)                # [64, L]
    sin_sgn = np.concatenate([sin, -sin], axis=0)             # [64, L]
    return cos64, sin_sgn


def _prep_weights(Wq, bq, Wk, bk, Wv, bv, Wo, bo):
    perm = np.concatenate([np.arange(0, D, 2), np.arange(1, D, 2)])
    WqP = np.asarray(Wq, dtype=np.float32)[:, :, perm].reshape(F, H * D)
    bqP = np.asarray(bq, dtype=np.float32)[:, perm].reshape(H * D)
    WkP = np.asarray(Wk, dtype=np.float32)[:, perm]
    bkP = np.asarray(bk, dtype=np.float32)[perm]
    Wkv = np.concatenate([WkP, np.asarray(Wv, dtype=np.float32)], axis=1)
    bkv = np.concatenate([bkP, np.asarray(bv, dtype=np.float32)])
    WoR = np.asarray(Wo, dtype=np.float32).reshape(H * D, F)
    bo_ = np.asarray(bo, dtype=np.float32)

    wq_pret = np.ascontiguousarray(
        WqP.reshape(FCH, 128, FCH, 128).transpose(2, 1, 0, 3)).astype(
        ml_dtypes.bfloat16)
    wkv_pret = np.ascontiguousarray(
        Wkv.reshape(FCH, 128, 128).transpose(1, 0, 2)).astype(
        ml_dtypes.bfloat16)
    wo_pret = np.ascontiguousarray(
        WoR.reshape(FCH, 128, FCH, 128).transpose(2, 1, 0, 3)).astype(
        ml_dtypes.bfloat16)
    bqbo = np.ascontiguousarray(np.concatenate(
        [bqP.reshape(FCH, 128).T, bo_.reshape(FCH, 128).T], axis=1))
    return wq_pret, wkv_pret, wo_pret, bqbo, bkv


def kernel(inputs_q, inputs_kv, mask, Wq, bq, Wk, bk, Wv, bv, Wo, bo):
    if "nc" not in _CACHED:
        _CACHED["nc"] = build_nc()
    nc = _CACHED["nc"]

    wq_pret, wkv_pret, wo_pret, bqbo, bkv = _prep_weights(
        Wq, bq, Wk, bk, Wv, bv, Wo, bo)

    cos64, sin_sgn = _tables()
    scale = 1.0 / np.sqrt(np.float32(D))
    cksk = np.ascontiguousarray(
        np.concatenate([cos64, sin_sgn], axis=1))      # [64, 2*L] (L=LK)
    cosq_full = np.tile(cos64 * scale, (2, 1))         # [128, L]
    sinq_full = np.tile(sin_sgn * scale, (2, 1))

    xq = np.asarray(inputs_q, dtype=np.float32)
    xkv = np.asarray(inputs_kv, dtype=np.float32)
    mk = np.asarray(mask)

    in_maps = []
    for core in range(NCORES):
        b = core // 4
        qs = (core % 4) * LQ
        xq_t = np.ascontiguousarray(
            xq[b, qs:qs + LQ, :].T.reshape(FCH, 128, LQ)).astype(
            ml_dtypes.bfloat16)
        xkv_t = np.ascontiguousarray(
            xkv[b].T.reshape(FCH, 128, NL, LQ).transpose(2, 0, 1, 3)).astype(
            ml_dtypes.bfloat16)
        mask_t = np.ascontiguousarray(
            mk[b, 0, qs:qs + LQ, :].T.reshape(KCH, 128, LQ)
            .astype(np.float16))
        in_maps.append({
            "xq_t": xq_t,
            "xkv_t": xkv_t,
            "mask_t": mask_t,
            "wq": wq_pret,
            "wkv": wkv_pret,
            "wo": wo_pret,
            "bqbo": bqbo,
            "bkv": bkv,
            "cosq": np.ascontiguousarray(cosq_full[:, qs:qs + LQ]),
            "sinq": np.ascontiguousarray(sinq_full[:, qs:qs + LQ]),
            "cksk": cksk,
        })

    res = bass_utils.run_bass_kernel_spmd(nc, in_maps,
                                          core_ids=list(range(NCORES)))
    _CACHED["last_results"] = res
    _CACHED["last_maps"] = in_maps

    out = np.empty((B, L, F), dtype=np.float32)
    for core in range(NCORES):
        b = core // 4
        qs = (core % 4) * LQ
        out[b, qs:qs + LQ, :] = res.results[core]["yT"].T
    return out
